# revision 78
# baseline (speedup 1.0000x reference)
"""Trainium2 Bass kernel for CelltypeDeconvolver (GCN message passing).

Runs SPMD on 8 NeuronCores. Nodes are partitioned across cores. Per GCN
layer each core computes h_pre = H @ W for its nodes (scaled by
dinv[src]); the dinv-scaled features are exchanged in three pipelined
AllGather window-groups (each fired as soon as its producer windows
finish, overlapping the next stage), landing in replicated per-group
DRAM tables. Edge source rows are then dma_gathered (software DGE,
8-block chunks rotated over the 4 swdge queues, sized so each chunk's
descriptors fit the 128-slot ring) and segment-reduced on the
TensorEngine with 0/1 fp8 selection matrices generated on-device
(is_equal against an iota tile). Self-loops are folded into the PSUM
accumulation via an identity matmul; the next stage's pre-matmul
(conv2 / decoder mlp1 + BN stats) is interleaved per window so the
BN AllReduce fires immediately when conv2 drains. Epilogue PSUM reads
run on the Activation engine to keep the Vector engine free for mask
generation. Graph structure (edge bucketing, degrees, padding) is
prepared host-side in numpy; all float math happens on-device.
"""

import contextlib
import ctypes
import os
import sys
import types

import numpy as np

for _p in ("/opt/trn_rl_repo",):
    if os.path.isdir(_p) and _p not in sys.path:
        sys.path.append(_p)

import ml_dtypes

import concourse.bass as bass
import concourse.bacc as bacc
import concourse.mybir as mybir
from concourse import library_config
from concourse.tile import TileContext
from concourse.bass_utils import run_bass_kernel_spmd

BF16 = mybir.dt.bfloat16
F32 = mybir.dt.float32
FP8 = mybir.dt.float8e4
I16 = mybir.dt.int16
AX = mybir.AluOpType
AFT = mybir.ActivationFunctionType

NCORES = 8
P = 128
BN_EPS = 1e-5
CB = 8             # gather/S chunk size in 128-slot blocks

TRACE = False
TRACE_KW = {}
LAST = {}
_CACHE = {}


def _pack_k(w, kpad):
    """[K, N] f32 -> [K2, 128, 2, N] bf16 packed (k = k2*256 + r*128 + p)."""
    w = np.asarray(w, np.float32)
    k, n = w.shape
    wp = np.zeros((kpad, n), np.float32)
    wp[:k] = w
    k2 = kpad // 256
    return np.ascontiguousarray(
        wp.reshape(k2, 2, P, n).transpose(0, 2, 1, 3)).astype(ml_dtypes.bfloat16)


def _cdiv(a, b):
    return (a + b - 1) // b


# ──────────────────────────────────────────────────────────────────────
# host-side plan: shard nodes, bucket edges, build index / S arrays
# ──────────────────────────────────────────────────────────────────────

def _plan(n, edge_index):
    NP = _cdiv(_cdiv(n, NCORES), P) * P        # nodes per core (multiple of 128)
    W = NP // P                                 # dst windows per core
    # src window groups: first fires its halo exchange earliest, so keep it
    # small; each group's table must stay int16-indexable (<= 32767 rows).
    g1 = _cdiv(W, 4)
    g3 = _cdiv(W, 6)
    GB = [0, g1, W - g3, W]                     # group bounds
    G = len(GB) - 1
    GW = [GB[i + 1] - GB[i] for i in range(G)]  # group widths
    assert all(NCORES * gw * P <= 32767 for gw in GW)

    src = np.asarray(edge_index[0], np.int64)
    dst = np.asarray(edge_index[1], np.int64)
    deg = np.bincount(dst, minlength=n).astype(np.float32) + 1.0
    dinv = (1.0 / np.sqrt(deg)).astype(np.float32)

    c_arr = dst // NP
    w_arr = (dst % NP) // P
    dl_arr = (dst % P).astype(np.int64)
    # src node -> (window group, row in that group's table)
    # group-g table layout = [(c p w), D] over that group's windows
    cs = src // NP
    ii = src % NP
    ws = ii // P
    psrc = ii % P
    grp = np.searchsorted(np.asarray(GB[1:]), ws, side="right").astype(np.int64)
    gw_arr = np.asarray(GW, np.int64)[grp]
    gb_arr = np.asarray(GB[:-1], np.int64)[grp]
    row = (cs * (P * gw_arr) + psrc * gw_arr + (ws - gb_arr)).astype(np.int64)

    order = np.lexsort((row, w_arr, c_arr, grp))
    c_s, w_s, h_s = c_arr[order], w_arr[order], grp[order]
    row_s, dl_s = row[order], dl_arr[order]

    key = ((h_s * NCORES + c_s) * W + w_s)
    cnt = np.bincount(key, minlength=G * NCORES * W).reshape(G, NCORES, W)
    starts = np.zeros(G * NCORES * W + 1, np.int64)
    np.cumsum(cnt.reshape(-1), out=starts[1:])

    # per-group per-window block counts and stream offsets
    BLK = [np.maximum(_cdiv(cnt[g].max(axis=0), P), 1) for g in range(G)]
    OFF = []
    for g in range(G):
        o = np.zeros(W + 1, np.int64)
        np.cumsum(BLK[g], out=o[1:])
        OFF.append(o)
    NST = [int(OFF[g][-1]) for g in range(G)]   # blocks per stream
    SBASE = [int(sum(NST[:g])) for g in range(G)]
    BTOT = int(sum(NST))
    MAXB = max(int(max(b.max() for b in BLK)), CB)

    gidx_list, dl_list = [], []
    for c in range(NCORES):
        fidx = np.zeros(BTOT * P, np.int16)
        fdl = np.full(BTOT * P, -1, np.int64)
        for h in range(G):
            for w in range(W):
                k = (h * NCORES + c) * W + w
                s0, s1 = starts[k], starts[k + 1]
                m = s1 - s0
                if m == 0:
                    continue
                base = (SBASE[h] + OFF[h][w]) * P
                fidx[base:base + m] = row_s[s0:s1].astype(np.int16)
                fdl[base:base + m] = dl_s[s0:s1]
        gidx_list.append(np.ascontiguousarray(
            np.tile(fidx.reshape(-1, 16).T, (NCORES, 1))))
        dl = np.full((P, BTOT, 1), -1.0, np.float32)
        pos = np.nonzero(fdl >= 0)[0]
        dl[pos % P, pos // P, 0] = fdl[pos]
        dl_list.append(dl.astype(ml_dtypes.bfloat16))

    dinv_t, valid_t = [], []
    for c in range(NCORES):
        g = c * NP + (np.arange(P)[:, None] + P * np.arange(W)[None, :])
        real = g < n
        dv = np.zeros((P, W), np.float32)
        dv[real] = dinv[g[real]]
        dinv_t.append(dv)
        valid_t.append(real.astype(np.float32))

    return dict(n=n, NP=NP, W=W, GB=GB, GW=GW,
                BLK=[[int(v) for v in b] for b in BLK],
                OFF=[[int(v) for v in o] for o in OFF],
                NST=NST, SBASE=SBASE, BTOT=BTOT, MAXB=MAXB,
                gidx=gidx_list, dl=dl_list,
                dinv=dinv_t, valid=valid_t)


# ──────────────────────────────────────────────────────────────────────
# device program
# ──────────────────────────────────────────────────────────────────────

def _build(ninv, NP, W, GB, GW, BLK, OFF, NST, SBASE, BTOT,
           MAXB, K2E, D, C):
    RG = [list(range(NCORES))]
    G = len(GW)
    nc = bacc.Bacc("TRN2", num_devices=NCORES, num_swdge_queues=4)

    xt_d = nc.dram_tensor("xt", [K2E, P, 2, NP], BF16, kind="ExternalInput")
    wlin_d = nc.dram_tensor("wlin", [K2E, P, 2, D], BF16, kind="ExternalInput")
    w1_d = nc.dram_tensor("w1", [1, P, 2, D], BF16, kind="ExternalInput")
    w2_d = nc.dram_tensor("w2", [1, P, 2, D], BF16, kind="ExternalInput")
    wm1_d = nc.dram_tensor("wm1", [1, P, 2, D], BF16, kind="ExternalInput")
    wm2_d = nc.dram_tensor("wm2", [1, P, 2, C], BF16, kind="ExternalInput")
    b1r_d = nc.dram_tensor("b1r", [P, D], F32, kind="ExternalInput")
    b2r_d = nc.dram_tensor("b2r", [P, D], F32, kind="ExternalInput")
    bcr_d = nc.dram_tensor("bcr", [P, C], F32, kind="ExternalInput")
    gam_d = nc.dram_tensor("gam", [P, 2], F32, kind="ExternalInput")
    bet_d = nc.dram_tensor("bet", [P, 2], F32, kind="ExternalInput")
    ident_d = nc.dram_tensor("ident", [P, P], BF16, kind="ExternalInput")
    gidx_d = nc.dram_tensor("gidx", [P, BTOT * 8], I16, kind="ExternalInput")
    dl_d = nc.dram_tensor("dl", [P, BTOT, 1], BF16, kind="ExternalInput")
    iota_d = nc.dram_tensor("iota", [P, MAXB, P], BF16, kind="ExternalInput")
    dinv_d = nc.dram_tensor("dinv", [P, W], F32, kind="ExternalInput")
    valid_d = nc.dram_tensor("valid", [P, W], F32, kind="ExternalInput")
    out_d = nc.dram_tensor("out", [P, W, C], F32, kind="ExternalOutput")

    ag_ins = [[nc.dram_tensor(f"ag_in{g}_{i}", [P, GW[g] * D], BF16)
               for g in range(G)] for i in range(2)]
    tables = [[nc.dram_tensor(f"table{g}_{i}", [NCORES * GW[g] * P, D], BF16,
                              addr_space="Shared") for g in range(G)]
              for i in range(2)]
    bn_in = nc.dram_tensor("bn_in", [P, 4], F32)
    bn_out = nc.dram_tensor("bn_out", [P, 4], F32, addr_space="Shared")

    CH = 7                                     # encoder windows per x-chunk

    with TileContext(nc) as tc, contextlib.ExitStack() as ctx:
        cp = ctx.enter_context(tc.tile_pool(name="const", bufs=1))
        big = ctx.enter_context(tc.tile_pool(name="big", bufs=2))
        htp = ctx.enter_context(tc.tile_pool(name="htp", bufs=2))

        nc.gpsimd.load_library(library_config.mlp)

        def cload(dram, shape, dtype, tag, src=None):
            t = cp.tile(shape, dtype, tag=tag, name=tag)
            nc.sync.dma_start(t[:], dram[:] if src is None else src)
            return t

        ident_t = cload(ident_d, [P, P], BF16, "ident")
        w1_t = cload(w1_d, [P, 2, D], BF16, "w1", src=w1_d[0])
        w2_t = cload(w2_d, [P, 2, D], BF16, "w2", src=w2_d[0])
        wm1_t = cload(wm1_d, [P, 2, D], BF16, "wm1", src=wm1_d[0])
        wm2_t = cload(wm2_d, [P, 2, C], BF16, "wm2", src=wm2_d[0])
        b1r_t = cload(b1r_d, [P, D], F32, "b1r")
        b2r_t = cload(b2r_d, [P, D], F32, "b2r")
        bcr_t = cload(bcr_d, [P, C], F32, "bcr")
        gam_t = cload(gam_d, [P, 2], F32, "gam")
        bet_t = cload(bet_d, [P, 2], F32, "bet")
        gidx_t = cload(gidx_d, [P, BTOT * 8], I16, "gidx")
        dl_t = cload(dl_d, [P, BTOT, 1], BF16, "dl")
        iota_t = cload(iota_d, [P, MAXB, P], BF16, "iota")
        dinv_t = cload(dinv_d, [P, W], F32, "dinv")
        valid_t = cload(valid_d, [P, W], F32, "valid")

        # persistent activations: ht slots rotate h0T -> h1T -> h2T -> h4T
        ht = [htp.tile([P, 2, NP], BF16, tag="ht", name=f"ht{i}")
              for i in range(3)]
        # hpre0 / hpre1 / h3 share one 2-deep rotation: h3 (layer-2 output)
        # reuses hpre0's buffer, whose last reader is layer 1's self-loop.
        hpre = [big.tile([P, W, D], BF16, tag="bigbuf", name=f"hpre{i}")
                for i in range(2)]
        h3 = big.tile([P, 2, NP], BF16, tag="bigbuf", name="h3")
        sumps = cp.tile([P, 2, W], F32, tag="sumps")
        sqps = cp.tile([P, 2, W], F32, tag="sqps")

        def send(li, g):
            nc.sync.dma_start(ag_ins[li][g][:],
                              hpre[li][:, GB[g]:GB[g + 1], :])
            nc.gpsimd.collective_compute(
                "AllGather", AX.bypass, ins=[ag_ins[li][g][:]],
                outs=[tables[li][g][:]], replica_groups=RG)

        send_at = {GB[g + 1] - 1: g for g in range(G)}

        # ── encoder: h0 = x @ lin_w (node-major) → transpose → ht[0],
        #    with conv1's pre-matmul interleaved per window
        with tc.tile_pool(name="encw", bufs=2) as wp, \
             tc.tile_pool(name="encp", bufs=2, space="PSUM") as pp, \
             tc.tile_pool(name="xtp", bufs=2) as xtp:
            wlin_t = []
            for k2 in range(K2E):
                t = cp.tile([P, 2, D], BF16, tag=f"wlin{k2}", name=f"wlin{k2}")
                nc.sync.dma_start(t[:], wlin_d[k2])
                wlin_t.append(t)
            for wc in range(_cdiv(W, CH)):
                ws, we = wc * CH, min(W, (wc + 1) * CH)
                xtc = []
                for k2 in range(K2E):
                    t = xtp.tile([P, 2, CH * P], BF16, tag=f"xtc{k2}",
                                 name=f"xtc{k2}_{wc}")
                    eng = nc.sync if k2 % 2 == 0 else nc.scalar
                    eng.dma_start(t[:, :, :(we - ws) * P],
                                  xt_d[k2][:, :, ws * P:we * P])
                    xtc.append(t)
                for w in range(ws, we):
                    lsl = slice((w - ws) * P, (w - ws + 1) * P)
                    sl = slice(w * P, (w + 1) * P)
                    ps = pp.tile([P, D], F32, tag="ps", name=f"eps{w}")
                    for k2 in range(K2E):
                        for r in range(2):
                            nc.tensor.matmul(
                                ps[:], xtc[k2][:, r, lsl], wlin_t[k2][:, r, :],
                                start=(k2 == 0 and r == 0),
                                stop=(k2 == K2E - 1 and r == 1))
                    hb = wp.tile([P, D], BF16, tag="hb", name=f"ehb{w}")
                    nc.vector.tensor_copy(hb[:], ps[:])
                    for r in range(2):
                        pt = pp.tile([P, P], BF16, tag="pt", name=f"ept{w}_{r}")
                        nc.tensor.transpose(pt[:], hb[:, r * P:(r + 1) * P],
                                            ident_t[:])
                        nc.vector.tensor_copy(ht[0][:, r, sl], pt[:])
                    ps2 = pp.tile([P, D], F32, tag="ps_pre", name=f"pre0_{w}")
                    for r in range(2):
                        nc.tensor.matmul(ps2[:], ht[0][:, r, sl], w1_t[:, r, :],
                                         start=(r == 0), stop=(r == 1))
                    nc.scalar.activation(hpre[0][:, w, :], ps2[:], AFT.Copy,
                                         scale=dinv_t[:, w:w + 1])
                    if w in send_at:
                        send(0, send_at[w])

        # ── conv layers (layer li consumes tableA/B[li]; the next stage's
        #    pre-matmul + halo send are interleaved into this layer's loop)
        for li in range(2):
            HT_out = ht[li + 1]
            br = b1r_t if li == 0 else b2r_t
            with tc.tile_pool(name=f"cw{li}", bufs=3) as wp, \
                 tc.tile_pool(name=f"cp{li}", bufs=2, space="PSUM") as pp:

                chunks = {}
                qc = [0]

                def _get_chunk(hs, ci, chunks=chunks, wp=wp, li=li):
                    key = (hs, ci)
                    if key in chunks:
                        return chunks[key]
                    nstream = NST[hs]
                    base_blk = SBASE[hs] + ci * CB
                    nblk = min(CB, nstream - ci * CB)
                    nn = nblk * P
                    gt = wp.tile([P, CB, D], BF16, tag=f"gt{hs}",
                                 name=f"gt{li}_{hs}_{ci}")
                    stt = wp.tile([P, CB, P], FP8, tag=f"st{hs}",
                                  name=f"st{li}_{hs}_{ci}")
                    tb = tables[li][hs][:]
                    nc.gpsimd.dma_gather(
                        gt[:, :nblk, :], tb,
                        gidx_t[:, base_blk * 8:(base_blk + nblk) * 8],
                        nn, nn, D, single_packet=True,
                        queue_num=qc[0] % 4)
                    qc[0] += 1
                    # build the 0/1 selection block on-device: S[e,b,j] =
                    # (j == dst_lane[e,b]); padding slots have dl = -1.
                    in0, in1 = bass.broadcast_tensor_aps(
                        iota_t[:, :nblk, :],
                        dl_t[:, base_blk:base_blk + nblk, :])
                    nc.vector.tensor_tensor(stt[:, :nblk, :], in0, in1,
                                            op=AX.is_equal)
                    chunks[key] = (gt, stt)
                    return chunks[key]

                def chunk_spans(off, nblk):
                    out = []
                    b = off
                    while b < off + nblk:
                        ci = b // CB
                        b1 = min(off + nblk, (ci + 1) * CB)
                        out.append((ci, b - ci * CB, b1 - ci * CB))
                        b = b1
                    return out

                for w in range(W):
                    sl = slice(w * P, (w + 1) * P)
                    pa = pp.tile([P, D], F32, tag="ps_agg", name=f"agg{li}_{w}")
                    spans = [(g, s) for g in range(G)
                             for s in chunk_spans(OFF[g][w], BLK[g][w])]
                    nmm = sum(s[2] - s[1] for _, s in spans)
                    # self-loop folded into psum: pa = hpre[w] + sum S.gt
                    nc.tensor.matmul(pa[:], ident_t[:], hpre[li][:, w, :],
                                     start=True, stop=False)
                    mi = 0
                    for hs, (ci, b0, b1) in spans:
                        gt, stt = _get_chunk(hs, ci)
                        for b in range(b0, b1):
                            nc.tensor.matmul(pa[:], stt[:, b, :], gt[:, b, :],
                                             start=False,
                                             stop=(mi == nmm - 1))
                            mi += 1
                    tf2 = wp.tile([P, D], F32, tag="tf2", name=f"tf2{li}_{w}")
                    nc.vector.scalar_tensor_tensor(
                        tf2[:], pa[:], dinv_t[:, w:w + 1], br[:],
                        op0=AX.mult, op1=AX.add)
                    hb = wp.tile([P, D], BF16, tag="hb2", name=f"chb{li}_{w}")
                    nc.scalar.activation(hb[:], tf2[:], AFT.Relu,
                                         scale=valid_t[:, w:w + 1])
                    for r in range(2):
                        pt = pp.tile([P, P], BF16, tag="pt",
                                     name=f"cpt{li}_{w}_{r}")
                        nc.tensor.transpose(pt[:], hb[:, r * P:(r + 1) * P],
                                            ident_t[:])
                        nc.vector.tensor_copy(HT_out[:, r, sl], pt[:])
                    if li == 0:
                        ps2 = pp.tile([P, D], F32, tag="ps_pre",
                                      name=f"pre1_{w}")
                        for r in range(2):
                            nc.tensor.matmul(ps2[:], HT_out[:, r, sl],
                                             w2_t[:, r, :],
                                             start=(r == 0), stop=(r == 1))
                        nc.scalar.activation(hpre[1][:, w, :], ps2[:], AFT.Copy,
                                             scale=dinv_t[:, w:w + 1])
                        if w in send_at:
                            send(1, send_at[w])
                    else:
                        # decoder mlp1 per window (feat-major) + BN stats
                        for fb in range(2):
                            pm = pp.tile([P, P], F32, tag=f"pm{fb}",
                                         name=f"pm{fb}_{w}")
                            for r in range(2):
                                nc.tensor.matmul(
                                    pm[:], wm1_t[:, r, fb * P:(fb + 1) * P],
                                    HT_out[:, r, sl],
                                    start=(r == 0), stop=(r == 1))
                            nc.vector.tensor_scalar(
                                h3[:, fb, sl], pm[:], 1.0, 0.0, op0=AX.mult,
                                op1=AX.add, accum_out=sumps[:, fb, w:w + 1])
                            scr = wp.tile([P, P], F32, tag=f"scr{fb}",
                                          name=f"scr{fb}_{w}")
                            nc.vector.scalar_tensor_tensor(
                                scr[:], h3[:, fb, sl], 1.0, h3[:, fb, sl],
                                op0=AX.mult, op1=AX.mult,
                                accum_out=sqps[:, fb, w:w + 1])

        # ── decoder: BN + relu + mlp2 + softmax (mlp1 ran inside layer 2)
        ht4 = htp.tile([P, 2, NP], BF16, tag="ht", name="ht4")
        with tc.tile_pool(name="dec", bufs=2) as wp, \
             tc.tile_pool(name="decp", bufs=2, space="PSUM") as pp, \
             tc.tile_pool(name="st1", bufs=1) as sp:
            sums = sp.tile([P, 2], F32, tag="sums")
            sqs = sp.tile([P, 2], F32, tag="sqs")
            for fb in range(2):
                nc.vector.reduce_sum(sums[:, fb:fb + 1], sumps[:, fb, :],
                                     axis=mybir.AxisListType.X)
                nc.vector.reduce_sum(sqs[:, fb:fb + 1], sqps[:, fb, :],
                                     axis=mybir.AxisListType.X)
            bnio = sp.tile([P, 4], F32, tag="bnio")
            nc.vector.tensor_copy(bnio[:, 0:2], sums[:])
            nc.vector.tensor_copy(bnio[:, 2:4], sqs[:])
            nc.sync.dma_start(bn_in[:], bnio[:])
            nc.gpsimd.collective_compute(
                "AllReduce", AX.add, ins=[bn_in[:]], outs=[bn_out[:]],
                replica_groups=RG)
            bns = sp.tile([P, 4], F32, tag="bns")
            nc.sync.dma_start(bns[:], bn_out[:])

            mu = sp.tile([P, 2], F32, tag="mu")
            nc.vector.tensor_scalar(mu[:], bns[:, 0:2], ninv, None, op0=AX.mult)
            msq = sp.tile([P, 2], F32, tag="msq")
            nc.vector.tensor_tensor(msq[:], mu[:], mu[:], op=AX.mult)
            var = sp.tile([P, 2], F32, tag="var")
            nc.vector.scalar_tensor_tensor(var[:], bns[:, 2:4], ninv, msq[:],
                                           op0=AX.mult, op1=AX.subtract)
            vae = sp.tile([P, 2], F32, tag="vae")
            nc.vector.tensor_scalar(vae[:], var[:], BN_EPS, None, op0=AX.add)
            sd = sp.tile([P, 2], F32, tag="sd")
            nc.scalar.activation(sd[:], vae[:], AFT.Sqrt)
            rstd = sp.tile([P, 2], F32, tag="rstd")
            nc.vector.reciprocal(rstd[:], sd[:])
            A = sp.tile([P, 2], F32, tag="A")
            nc.vector.tensor_tensor(A[:], rstd[:], gam_t[:], op=AX.mult)
            tb = sp.tile([P, 2], F32, tag="tb")
            nc.vector.tensor_tensor(tb[:], mu[:], A[:], op=AX.mult)
            B = sp.tile([P, 2], F32, tag="B")
            nc.vector.tensor_tensor(B[:], bet_t[:], tb[:], op=AX.subtract)

            lg = sp.tile([P, W, C], F32, tag="lg")
            ex = sp.tile([P, W, C], F32, tag="ex")
            rs = sp.tile([P, W], F32, tag="rs")
            ri = sp.tile([P, W], F32, tag="ri")
            outst = sp.tile([P, W, C], F32, tag="outst")
            TW = 12
            for wc0 in range(0, W, TW):
                wc1 = min(W, wc0 + TW)
                csl = slice(wc0 * P, wc1 * P)
                for fb in range(2):
                    nc.scalar.activation(ht4[:, fb, csl], h3[:, fb, csl],
                                         AFT.Relu, bias=B[:, fb:fb + 1],
                                         scale=A[:, fb:fb + 1])
                for w in range(wc0, wc1):
                    sl = slice(w * P, (w + 1) * P)
                    pl = pp.tile([P, C], F32, tag="ps_lg", name=f"plg{w}")
                    for r in range(2):
                        nc.tensor.matmul(pl[:], ht4[:, r, sl], wm2_t[:, r, :],
                                         start=(r == 0), stop=(r == 1))
                    nc.vector.scalar_tensor_tensor(lg[:, w, :], pl[:], 1.0,
                                                   bcr_t[:],
                                                   op0=AX.mult, op1=AX.add)
                nc.scalar.activation(
                    ex[:, wc0:wc1, :].rearrange("p w c -> p (w c)"),
                    lg[:, wc0:wc1, :].rearrange("p w c -> p (w c)"), AFT.Exp)
                nc.vector.reduce_sum(rs[:, wc0:wc1], ex[:, wc0:wc1, :],
                                     axis=mybir.AxisListType.X)
                nc.vector.reciprocal(ri[:, wc0:wc1], rs[:, wc0:wc1])
                for w in range(wc0, wc1):
                    nc.vector.tensor_scalar(outst[:, w, :], ex[:, w, :],
                                            ri[:, w:w + 1], None, op0=AX.mult)
                nc.sync.dma_start(out_d[:, wc0:wc1, :], outst[:, wc0:wc1, :])

    nc.compile()
    return nc


# ──────────────────────────────────────────────────────────────────────
# NTFF profiling shim (only needed when TRACE)
# ──────────────────────────────────────────────────────────────────────

def _install_hook():
    if "antenv.axon_hooks" in sys.modules:
        return
    so_path = "/opt/axon/libaxon_pjrt.so"
    holder = {"hook": None}
    mod = types.ModuleType("antenv.axon_hooks")
    mod.set_axon_ntff_profile_hook = lambda h: holder.__setitem__("hook", h)
    mod.get_axon_ntff_profile_hook = lambda: holder["hook"]
    sys.modules["antenv.axon_hooks"] = mod
    try:
        import antenv
        antenv.axon_hooks = mod
    except ImportError:
        pass
    try:
        lib = ctypes.CDLL(so_path)
        lib.axon_start_nrt_profile.argtypes = [ctypes.POINTER(ctypes.c_int64),
                                               ctypes.c_size_t]
        lib.axon_start_nrt_profile.restype = ctypes.c_int64
        lib.axon_stop_nrt_profile.argtypes = [ctypes.c_char_p]
        lib.axon_stop_nrt_profile.restype = ctypes.c_int64

        @contextlib.contextmanager
        def _hook(output_dir, device_ids):
            import jax
            jax.devices()
            if device_ids:
                ids = (ctypes.c_int64 * len(device_ids))(*device_ids)
                rc = lib.axon_start_nrt_profile(ids, len(device_ids))
            else:
                rc = lib.axon_start_nrt_profile(None, 0)
            if rc != 0:
                raise RuntimeError(f"axon_start_nrt_profile rc={rc}")
            try:
                yield
            finally:
                nf = lib.axon_stop_nrt_profile(str(output_dir).encode())
                if nf < 0:
                    raise RuntimeError(f"axon_stop_nrt_profile rc={nf}")

        holder["hook"] = _hook
    except OSError:
        pass


# ──────────────────────────────────────────────────────────────────────
# entry point
# ──────────────────────────────────────────────────────────────────────

def kernel(x, edge_index, lin_w, conv1_w, conv1_b, conv2_w, conv2_b,
           mlp1_w, mlp1_b, bn_gamma, bn_beta, mlp2_w, mlp2_b):
    x = np.asarray(x, np.float32)
    n, g = x.shape
    D = int(np.asarray(lin_w).shape[1])
    C = int(np.asarray(mlp2_w).shape[1])
    KENC = _cdiv(g, 256) * 256
    K2E = KENC // 256

    plan = _plan(n, np.asarray(edge_index))
    NP, W, BTOT = plan["NP"], plan["W"], plan["BTOT"]

    key = (n, g, D, C, NP,
           tuple(tuple(b) for b in plan["BLK"]), tuple(plan["GB"]))
    if key not in _CACHE:
        _CACHE[key] = _build(1.0 / float(n), NP, W, plan["GB"], plan["GW"],
                             plan["BLK"], plan["OFF"], plan["NST"],
                             plan["SBASE"], BTOT, plan["MAXB"],
                             K2E, D, C)
    nc = _CACHE[key]

    shared = {
        "wlin": _pack_k(lin_w, KENC),
        "w1": _pack_k(conv1_w, D),
        "w2": _pack_k(conv2_w, D),
        "wm1": _pack_k(mlp1_w, D),
        "wm2": _pack_k(mlp2_w, D),
        "b1r": np.ascontiguousarray(
            np.broadcast_to(np.asarray(conv1_b, np.float32), (P, D))),
        "b2r": np.ascontiguousarray(
            np.broadcast_to(np.asarray(conv2_b, np.float32), (P, D))),
        "bcr": np.ascontiguousarray(
            np.broadcast_to(np.asarray(mlp2_b, np.float32), (P, C))),
        "gam": np.ascontiguousarray(
            np.asarray(bn_gamma, np.float32).reshape(2, P).T),
        "bet": np.ascontiguousarray(
            np.asarray(bn_beta, np.float32).reshape(2, P).T),
        "ident": np.eye(P, dtype=np.float32).astype(ml_dtypes.bfloat16),
        "iota": np.ascontiguousarray(np.broadcast_to(
            np.arange(P, dtype=np.float32), (P, plan["MAXB"], P))).astype(
                ml_dtypes.bfloat16),
    }

    in_maps = []
    for c in range(NCORES):
        xs = x[c * NP:(c + 1) * NP]
        if xs.shape[0] < NP:
            xs = np.vstack([xs, np.zeros((NP - xs.shape[0], g), np.float32)])
        xt = _pack_k(np.ascontiguousarray(xs.T), KENC)
        in_maps.append(dict(shared,
                            xt=xt,
                            gidx=plan["gidx"][c],
                            dl=plan["dl"][c],
                            dinv=plan["dinv"][c],
                            valid=plan["valid"][c]))

    if TRACE:
        _install_hook()
        res = run_bass_kernel_spmd(nc, in_maps, core_ids=list(range(NCORES)),
                                   trace=True, **TRACE_KW)
        LAST["exec_time_ns"] = res.exec_time_ns
        LAST["res"] = res
    else:
        res = run_bass_kernel_spmd(nc, in_maps, core_ids=list(range(NCORES)))

    parts = []
    for c in range(NCORES):
        o = np.asarray(res.results[c]["out"])            # [P, W, C]
        parts.append(np.ascontiguousarray(o.transpose(1, 0, 2)).reshape(NP, C))
    return np.concatenate(parts, axis=0)[:n].astype(np.float32)



# revision 79
# speedup vs baseline: 1.2687x; 1.2687x over previous
"""Trainium2 Bass kernel for CelltypeDeconvolver (GCN message passing).

Runs SPMD on 8 NeuronCores. Nodes are partitioned across cores. Per GCN
layer each core computes h_pre = H @ W for its nodes (scaled by
dinv[src]); the dinv-scaled features are exchanged in three pipelined
AllGather window-groups (each fired as soon as its producer windows
finish, overlapping the next stage), landing in replicated per-group
DRAM tables. Edge source rows are then dma_gathered (software DGE,
8-block chunks rotated over the 4 swdge queues, sized so each chunk's
descriptors fit the 128-slot ring) and segment-reduced on the
TensorEngine with 0/1 fp8 selection matrices generated on-device
(is_equal against an iota tile). Self-loops are folded into the PSUM
accumulation via an identity matmul; the next stage's pre-matmul
(conv2 / decoder mlp1 + BN stats) is interleaved per window so the
BN AllReduce fires immediately when conv2 drains. Epilogue PSUM reads
run on the Activation engine to keep the Vector engine free for mask
generation. Graph structure (edge bucketing, degrees, padding) is
prepared host-side in numpy; all float math happens on-device.
"""

import contextlib
import ctypes
import os
import sys
import types

import numpy as np

for _p in ("/opt/trn_rl_repo",):
    if os.path.isdir(_p) and _p not in sys.path:
        sys.path.append(_p)

import ml_dtypes

import concourse.bass as bass
import concourse.bacc as bacc
import concourse.mybir as mybir
from concourse import library_config
from concourse.tile import TileContext
from concourse.bass_utils import run_bass_kernel_spmd

BF16 = mybir.dt.bfloat16
F32 = mybir.dt.float32
FP8 = mybir.dt.float8e4
I16 = mybir.dt.int16
AX = mybir.AluOpType
AFT = mybir.ActivationFunctionType

NCORES = 8
P = 128
BN_EPS = 1e-5
CB = 8             # gather/S chunk size in 128-slot blocks

TRACE = False
TRACE_KW = {}
LAST = {}
_CACHE = {}


def _pack_k(w, kpad):
    """[K, N] f32 -> [K2, 128, 2, N] bf16 packed (k = k2*256 + r*128 + p)."""
    w = np.asarray(w, np.float32)
    k, n = w.shape
    wp = np.zeros((kpad, n), np.float32)
    wp[:k] = w
    k2 = kpad // 256
    return np.ascontiguousarray(
        wp.reshape(k2, 2, P, n).transpose(0, 2, 1, 3)).astype(ml_dtypes.bfloat16)


def _cdiv(a, b):
    return (a + b - 1) // b


# ──────────────────────────────────────────────────────────────────────
# host-side plan: shard nodes, bucket edges, build index / S arrays
# ──────────────────────────────────────────────────────────────────────

def _plan(n, edge_index):
    NP = _cdiv(_cdiv(n, NCORES), P) * P        # nodes per core (multiple of 128)
    W = NP // P                                 # dst windows per core
    # src window groups: first fires its halo exchange earliest, so keep it
    # small; each group's table must stay int16-indexable (<= 32767 rows).
    g1 = _cdiv(W, 4)
    g3 = _cdiv(W, 4)
    GB = [0, g1, W - g3, W]                     # group bounds
    G = len(GB) - 1
    GW = [GB[i + 1] - GB[i] for i in range(G)]  # group widths
    assert all(NCORES * gw * P <= 32767 for gw in GW)

    src = np.asarray(edge_index[0], np.int64)
    dst = np.asarray(edge_index[1], np.int64)
    deg = np.bincount(dst, minlength=n).astype(np.float32) + 1.0
    dinv = (1.0 / np.sqrt(deg)).astype(np.float32)

    c_arr = dst // NP
    w_arr = (dst % NP) // P
    dl_arr = (dst % P).astype(np.int64)
    # src node -> (window group, row in that group's table)
    # group-g table layout = [(c p w), D] over that group's windows
    cs = src // NP
    ii = src % NP
    ws = ii // P
    psrc = ii % P
    grp = np.searchsorted(np.asarray(GB[1:]), ws, side="right").astype(np.int64)
    gw_arr = np.asarray(GW, np.int64)[grp]
    gb_arr = np.asarray(GB[:-1], np.int64)[grp]
    row = (cs * (P * gw_arr) + psrc * gw_arr + (ws - gb_arr)).astype(np.int64)

    order = np.lexsort((row, w_arr, c_arr, grp))
    c_s, w_s, h_s = c_arr[order], w_arr[order], grp[order]
    row_s, dl_s = row[order], dl_arr[order]

    key = ((h_s * NCORES + c_s) * W + w_s)
    cnt = np.bincount(key, minlength=G * NCORES * W).reshape(G, NCORES, W)
    starts = np.zeros(G * NCORES * W + 1, np.int64)
    np.cumsum(cnt.reshape(-1), out=starts[1:])

    # per-group per-window block counts and stream offsets
    BLK = [np.maximum(_cdiv(cnt[g].max(axis=0), P), 1) for g in range(G)]
    OFF = []
    for g in range(G):
        o = np.zeros(W + 1, np.int64)
        np.cumsum(BLK[g], out=o[1:])
        OFF.append(o)
    NST = [int(OFF[g][-1]) for g in range(G)]   # blocks per stream
    SBASE = [int(sum(NST[:g])) for g in range(G)]
    BTOT = int(sum(NST))
    MAXB = max(int(max(b.max() for b in BLK)), CB)

    gidx_list, dl_list = [], []
    for c in range(NCORES):
        fidx = np.zeros(BTOT * P, np.int16)
        fdl = np.full(BTOT * P, -1, np.int64)
        for h in range(G):
            for w in range(W):
                k = (h * NCORES + c) * W + w
                s0, s1 = starts[k], starts[k + 1]
                m = s1 - s0
                if m == 0:
                    continue
                base = (SBASE[h] + OFF[h][w]) * P
                fidx[base:base + m] = row_s[s0:s1].astype(np.int16)
                fdl[base:base + m] = dl_s[s0:s1]
        gidx_list.append(np.ascontiguousarray(
            np.tile(fidx.reshape(-1, 16).T, (NCORES, 1))))
        dl = np.full((P, BTOT, 1), -1.0, np.float32)
        pos = np.nonzero(fdl >= 0)[0]
        dl[pos % P, pos // P, 0] = fdl[pos]
        dl_list.append(dl.astype(ml_dtypes.bfloat16))

    dinv_t, valid_t = [], []
    for c in range(NCORES):
        g = c * NP + (np.arange(P)[:, None] + P * np.arange(W)[None, :])
        real = g < n
        dv = np.zeros((P, W), np.float32)
        dv[real] = dinv[g[real]]
        dinv_t.append(dv)
        valid_t.append(real.astype(np.float32))

    return dict(n=n, NP=NP, W=W, GB=GB, GW=GW,
                BLK=[[int(v) for v in b] for b in BLK],
                OFF=[[int(v) for v in o] for o in OFF],
                NST=NST, SBASE=SBASE, BTOT=BTOT, MAXB=MAXB,
                gidx=gidx_list, dl=dl_list,
                dinv=dinv_t, valid=valid_t)


# ──────────────────────────────────────────────────────────────────────
# device program
# ──────────────────────────────────────────────────────────────────────

def _build(ninv, NP, W, GB, GW, BLK, OFF, NST, SBASE, BTOT,
           MAXB, K2E, D, C):
    RG = [list(range(NCORES))]
    G = len(GW)
    nc = bacc.Bacc("TRN2", num_devices=NCORES, num_swdge_queues=4)

    xt_d = nc.dram_tensor("xt", [K2E, P, 2, NP], BF16, kind="ExternalInput")
    wlin_d = nc.dram_tensor("wlin", [K2E, P, 2, D], BF16, kind="ExternalInput")
    w1_d = nc.dram_tensor("w1", [1, P, 2, D], BF16, kind="ExternalInput")
    w2_d = nc.dram_tensor("w2", [1, P, 2, D], BF16, kind="ExternalInput")
    wm1_d = nc.dram_tensor("wm1", [1, P, 2, D], BF16, kind="ExternalInput")
    wm2_d = nc.dram_tensor("wm2", [1, P, 2, C], BF16, kind="ExternalInput")
    b1r_d = nc.dram_tensor("b1r", [P, D], F32, kind="ExternalInput")
    b2r_d = nc.dram_tensor("b2r", [P, D], F32, kind="ExternalInput")
    bcr_d = nc.dram_tensor("bcr", [P, C], F32, kind="ExternalInput")
    gam_d = nc.dram_tensor("gam", [P, 2], F32, kind="ExternalInput")
    bet_d = nc.dram_tensor("bet", [P, 2], F32, kind="ExternalInput")
    ident_d = nc.dram_tensor("ident", [P, P], BF16, kind="ExternalInput")
    gidx_d = nc.dram_tensor("gidx", [P, BTOT * 8], I16, kind="ExternalInput")
    dl_d = nc.dram_tensor("dl", [P, BTOT, 1], BF16, kind="ExternalInput")
    iota_d = nc.dram_tensor("iota", [P, MAXB, P], BF16, kind="ExternalInput")
    dinv_d = nc.dram_tensor("dinv", [P, W], F32, kind="ExternalInput")
    valid_d = nc.dram_tensor("valid", [P, W], F32, kind="ExternalInput")
    out_d = nc.dram_tensor("out", [P, W, C], F32, kind="ExternalOutput")

    ag_ins = [[nc.dram_tensor(f"ag_in{g}_{i}", [P, GW[g] * D], BF16)
               for g in range(G)] for i in range(2)]
    tables = [[nc.dram_tensor(f"table{g}_{i}", [NCORES * GW[g] * P, D], BF16,
                              addr_space="Shared") for g in range(G)]
              for i in range(2)]
    bn_in = nc.dram_tensor("bn_in", [P, 4], F32)
    bn_out = nc.dram_tensor("bn_out", [P, 4], F32, addr_space="Shared")

    CH = 7                                     # encoder windows per x-chunk

    with TileContext(nc) as tc, contextlib.ExitStack() as ctx:
        cp = ctx.enter_context(tc.tile_pool(name="const", bufs=1))
        big = ctx.enter_context(tc.tile_pool(name="big", bufs=2))
        htp = ctx.enter_context(tc.tile_pool(name="htp", bufs=2))

        nc.gpsimd.load_library(library_config.mlp)

        def cload(dram, shape, dtype, tag, src=None):
            t = cp.tile(shape, dtype, tag=tag, name=tag)
            nc.sync.dma_start(t[:], dram[:] if src is None else src)
            return t

        ident_t = cload(ident_d, [P, P], BF16, "ident")
        w1_t = cload(w1_d, [P, 2, D], BF16, "w1", src=w1_d[0])
        w2_t = cload(w2_d, [P, 2, D], BF16, "w2", src=w2_d[0])
        wm1_t = cload(wm1_d, [P, 2, D], BF16, "wm1", src=wm1_d[0])
        wm2_t = cload(wm2_d, [P, 2, C], BF16, "wm2", src=wm2_d[0])
        b1r_t = cload(b1r_d, [P, D], F32, "b1r")
        b2r_t = cload(b2r_d, [P, D], F32, "b2r")
        bcr_t = cload(bcr_d, [P, C], F32, "bcr")
        gam_t = cload(gam_d, [P, 2], F32, "gam")
        bet_t = cload(bet_d, [P, 2], F32, "bet")
        gidx_t = cload(gidx_d, [P, BTOT * 8], I16, "gidx")
        dl_t = cload(dl_d, [P, BTOT, 1], BF16, "dl")
        iota_t = cload(iota_d, [P, MAXB, P], BF16, "iota")
        dinv_t = cload(dinv_d, [P, W], F32, "dinv")
        valid_t = cload(valid_d, [P, W], F32, "valid")

        # persistent activations: ht slots rotate h0T -> h1T -> h2T -> h4T
        ht = [htp.tile([P, 2, NP], BF16, tag="ht", name=f"ht{i}")
              for i in range(3)]
        # hpre0 / hpre1 / h3 share one 2-deep rotation: h3 (layer-2 output)
        # reuses hpre0's buffer, whose last reader is layer 1's self-loop.
        hpre = [big.tile([P, W, D], BF16, tag="bigbuf", name=f"hpre{i}")
                for i in range(2)]
        h3 = big.tile([P, 2, NP], BF16, tag="bigbuf", name="h3")
        sumps = cp.tile([P, 2, W], F32, tag="sumps")
        sqps = cp.tile([P, 2, W], F32, tag="sqps")

        def send(li, g):
            nc.sync.dma_start(ag_ins[li][g][:],
                              hpre[li][:, GB[g]:GB[g + 1], :])
            nc.gpsimd.collective_compute(
                "AllGather", AX.bypass, ins=[ag_ins[li][g][:]],
                outs=[tables[li][g][:]], replica_groups=RG)

        send_at = {GB[g + 1] - 1: g for g in range(G)}

        # ── encoder: h0 = x @ lin_w (node-major) → transpose → ht[0],
        #    with conv1's pre-matmul interleaved per window
        with tc.tile_pool(name="encw", bufs=2) as wp, \
             tc.tile_pool(name="encp", bufs=2, space="PSUM") as pp, \
             tc.tile_pool(name="xtp", bufs=2) as xtp:
            wlin_t = []
            for k2 in range(K2E):
                t = cp.tile([P, 2, D], BF16, tag=f"wlin{k2}", name=f"wlin{k2}")
                nc.sync.dma_start(t[:], wlin_d[k2])
                wlin_t.append(t)
            for wc in range(_cdiv(W, CH)):
                ws, we = wc * CH, min(W, (wc + 1) * CH)
                xtc = []
                for k2 in range(K2E):
                    t = xtp.tile([P, 2, CH * P], BF16, tag=f"xtc{k2}",
                                 name=f"xtc{k2}_{wc}")
                    eng = nc.sync if k2 % 2 == 0 else nc.scalar
                    eng.dma_start(t[:, :, :(we - ws) * P],
                                  xt_d[k2][:, :, ws * P:we * P])
                    xtc.append(t)
                for w in range(ws, we):
                    lsl = slice((w - ws) * P, (w - ws + 1) * P)
                    sl = slice(w * P, (w + 1) * P)
                    ps = pp.tile([P, D], F32, tag="ps", name=f"eps{w}")
                    for k2 in range(K2E):
                        for r in range(2):
                            nc.tensor.matmul(
                                ps[:], xtc[k2][:, r, lsl], wlin_t[k2][:, r, :],
                                start=(k2 == 0 and r == 0),
                                stop=(k2 == K2E - 1 and r == 1))
                    hb = wp.tile([P, D], BF16, tag="hb", name=f"ehb{w}")
                    nc.vector.tensor_copy(hb[:], ps[:])
                    for r in range(2):
                        pt = pp.tile([P, P], BF16, tag="pt", name=f"ept{w}_{r}")
                        nc.tensor.transpose(pt[:], hb[:, r * P:(r + 1) * P],
                                            ident_t[:])
                        nc.vector.tensor_copy(ht[0][:, r, sl], pt[:])
                    ps2 = pp.tile([P, D], F32, tag="ps_pre", name=f"pre0_{w}")
                    for r in range(2):
                        nc.tensor.matmul(ps2[:], ht[0][:, r, sl], w1_t[:, r, :],
                                         start=(r == 0), stop=(r == 1))
                    nc.scalar.activation(hpre[0][:, w, :], ps2[:], AFT.Copy,
                                         scale=dinv_t[:, w:w + 1])
                    if w in send_at:
                        send(0, send_at[w])

        # ── conv layers (layer li consumes tableA/B[li]; the next stage's
        #    pre-matmul + halo send are interleaved into this layer's loop)
        for li in range(2):
            HT_out = ht[li + 1]
            br = b1r_t if li == 0 else b2r_t
            with tc.tile_pool(name=f"cw{li}", bufs=3) as wp, \
                 tc.tile_pool(name=f"cp{li}", bufs=2, space="PSUM") as pp:

                chunks = {}
                qc = [0]

                def _get_chunk(hs, ci, chunks=chunks, wp=wp, li=li):
                    key = (hs, ci)
                    if key in chunks:
                        return chunks[key]
                    nstream = NST[hs]
                    base_blk = SBASE[hs] + ci * CB
                    nblk = min(CB, nstream - ci * CB)
                    nn = nblk * P
                    gt = wp.tile([P, CB, D], BF16, tag=f"gt{hs}",
                                 name=f"gt{li}_{hs}_{ci}")
                    stt = wp.tile([P, CB, P], FP8, tag=f"st{hs}",
                                  name=f"st{li}_{hs}_{ci}")
                    tb = tables[li][hs][:]
                    nc.gpsimd.dma_gather(
                        gt[:, :nblk, :], tb,
                        gidx_t[:, base_blk * 8:(base_blk + nblk) * 8],
                        nn, nn, D, single_packet=True,
                        queue_num=qc[0] % 4)
                    qc[0] += 1
                    # build the 0/1 selection block on-device: S[e,b,j] =
                    # (j == dst_lane[e,b]); padding slots have dl = -1.
                    in0, in1 = bass.broadcast_tensor_aps(
                        iota_t[:, :nblk, :],
                        dl_t[:, base_blk:base_blk + nblk, :])
                    nc.vector.tensor_tensor(stt[:, :nblk, :], in0, in1,
                                            op=AX.is_equal)
                    chunks[key] = (gt, stt)
                    return chunks[key]

                def chunk_spans(off, nblk):
                    out = []
                    b = off
                    while b < off + nblk:
                        ci = b // CB
                        b1 = min(off + nblk, (ci + 1) * CB)
                        out.append((ci, b - ci * CB, b1 - ci * CB))
                        b = b1
                    return out

                for w in range(W):
                    sl = slice(w * P, (w + 1) * P)
                    pa = pp.tile([P, D], F32, tag="ps_agg", name=f"agg{li}_{w}")
                    spans = [(g, s) for g in range(G)
                             for s in chunk_spans(OFF[g][w], BLK[g][w])]
                    nmm = sum(s[2] - s[1] for _, s in spans)
                    # self-loop folded into psum: pa = hpre[w] + sum S.gt
                    nc.tensor.matmul(pa[:], ident_t[:], hpre[li][:, w, :],
                                     start=True, stop=False)
                    mi = 0
                    for hs, (ci, b0, b1) in spans:
                        gt, stt = _get_chunk(hs, ci)
                        for b in range(b0, b1):
                            nc.tensor.matmul(pa[:], stt[:, b, :], gt[:, b, :],
                                             start=False,
                                             stop=(mi == nmm - 1))
                            mi += 1
                    tf2 = wp.tile([P, D], F32, tag="tf2", name=f"tf2{li}_{w}")
                    nc.vector.scalar_tensor_tensor(
                        tf2[:], pa[:], dinv_t[:, w:w + 1], br[:],
                        op0=AX.mult, op1=AX.add)
                    hb = wp.tile([P, D], BF16, tag="hb2", name=f"chb{li}_{w}")
                    nc.scalar.activation(hb[:], tf2[:], AFT.Relu,
                                         scale=valid_t[:, w:w + 1])
                    for r in range(2):
                        pt = pp.tile([P, P], BF16, tag="pt",
                                     name=f"cpt{li}_{w}_{r}")
                        nc.tensor.transpose(pt[:], hb[:, r * P:(r + 1) * P],
                                            ident_t[:])
                        nc.vector.tensor_copy(HT_out[:, r, sl], pt[:])
                    if li == 0:
                        ps2 = pp.tile([P, D], F32, tag="ps_pre",
                                      name=f"pre1_{w}")
                        for r in range(2):
                            nc.tensor.matmul(ps2[:], HT_out[:, r, sl],
                                             w2_t[:, r, :],
                                             start=(r == 0), stop=(r == 1))
                        nc.scalar.activation(hpre[1][:, w, :], ps2[:], AFT.Copy,
                                             scale=dinv_t[:, w:w + 1])
                        if w in send_at:
                            send(1, send_at[w])
                    else:
                        # decoder mlp1 per window (feat-major) + BN stats
                        for fb in range(2):
                            pm = pp.tile([P, P], F32, tag=f"pm{fb}",
                                         name=f"pm{fb}_{w}")
                            for r in range(2):
                                nc.tensor.matmul(
                                    pm[:], wm1_t[:, r, fb * P:(fb + 1) * P],
                                    HT_out[:, r, sl],
                                    start=(r == 0), stop=(r == 1))
                            nc.vector.tensor_scalar(
                                h3[:, fb, sl], pm[:], 1.0, 0.0, op0=AX.mult,
                                op1=AX.add, accum_out=sumps[:, fb, w:w + 1])
                            scr = wp.tile([P, P], F32, tag=f"scr{fb}",
                                          name=f"scr{fb}_{w}")
                            nc.vector.scalar_tensor_tensor(
                                scr[:], h3[:, fb, sl], 1.0, h3[:, fb, sl],
                                op0=AX.mult, op1=AX.mult,
                                accum_out=sqps[:, fb, w:w + 1])

        # ── decoder: BN + relu + mlp2 + softmax (mlp1 ran inside layer 2)
        ht4 = htp.tile([P, 2, NP], BF16, tag="ht", name="ht4")
        with tc.tile_pool(name="dec", bufs=2) as wp, \
             tc.tile_pool(name="decp", bufs=2, space="PSUM") as pp, \
             tc.tile_pool(name="st1", bufs=1) as sp:
            sums = sp.tile([P, 2], F32, tag="sums")
            sqs = sp.tile([P, 2], F32, tag="sqs")
            for fb in range(2):
                nc.vector.reduce_sum(sums[:, fb:fb + 1], sumps[:, fb, :],
                                     axis=mybir.AxisListType.X)
                nc.vector.reduce_sum(sqs[:, fb:fb + 1], sqps[:, fb, :],
                                     axis=mybir.AxisListType.X)
            bnio = sp.tile([P, 4], F32, tag="bnio")
            nc.vector.tensor_copy(bnio[:, 0:2], sums[:])
            nc.vector.tensor_copy(bnio[:, 2:4], sqs[:])
            nc.sync.dma_start(bn_in[:], bnio[:])
            nc.gpsimd.collective_compute(
                "AllReduce", AX.add, ins=[bn_in[:]], outs=[bn_out[:]],
                replica_groups=RG)
            bns = sp.tile([P, 4], F32, tag="bns")
            nc.sync.dma_start(bns[:], bn_out[:])

            mu = sp.tile([P, 2], F32, tag="mu")
            nc.vector.tensor_scalar(mu[:], bns[:, 0:2], ninv, None, op0=AX.mult)
            msq = sp.tile([P, 2], F32, tag="msq")
            nc.vector.tensor_tensor(msq[:], mu[:], mu[:], op=AX.mult)
            var = sp.tile([P, 2], F32, tag="var")
            nc.vector.scalar_tensor_tensor(var[:], bns[:, 2:4], ninv, msq[:],
                                           op0=AX.mult, op1=AX.subtract)
            vae = sp.tile([P, 2], F32, tag="vae")
            nc.vector.tensor_scalar(vae[:], var[:], BN_EPS, None, op0=AX.add)
            sd = sp.tile([P, 2], F32, tag="sd")
            nc.scalar.activation(sd[:], vae[:], AFT.Sqrt)
            rstd = sp.tile([P, 2], F32, tag="rstd")
            nc.vector.reciprocal(rstd[:], sd[:])
            A = sp.tile([P, 2], F32, tag="A")
            nc.vector.tensor_tensor(A[:], rstd[:], gam_t[:], op=AX.mult)
            tb = sp.tile([P, 2], F32, tag="tb")
            nc.vector.tensor_tensor(tb[:], mu[:], A[:], op=AX.mult)
            B = sp.tile([P, 2], F32, tag="B")
            nc.vector.tensor_tensor(B[:], bet_t[:], tb[:], op=AX.subtract)

            lg = sp.tile([P, W, C], F32, tag="lg")
            ex = sp.tile([P, W, C], F32, tag="ex")
            rs = sp.tile([P, W], F32, tag="rs")
            ri = sp.tile([P, W], F32, tag="ri")
            outst = sp.tile([P, W, C], F32, tag="outst")
            TW = 12
            for wc0 in range(0, W, TW):
                wc1 = min(W, wc0 + TW)
                csl = slice(wc0 * P, wc1 * P)
                for fb in range(2):
                    nc.scalar.activation(ht4[:, fb, csl], h3[:, fb, csl],
                                         AFT.Relu, bias=B[:, fb:fb + 1],
                                         scale=A[:, fb:fb + 1])
                for w in range(wc0, wc1):
                    sl = slice(w * P, (w + 1) * P)
                    pl = pp.tile([P, C], F32, tag="ps_lg", name=f"plg{w}")
                    for r in range(2):
                        nc.tensor.matmul(pl[:], ht4[:, r, sl], wm2_t[:, r, :],
                                         start=(r == 0), stop=(r == 1))
                    nc.vector.scalar_tensor_tensor(lg[:, w, :], pl[:], 1.0,
                                                   bcr_t[:],
                                                   op0=AX.mult, op1=AX.add)
                nc.scalar.activation(
                    ex[:, wc0:wc1, :].rearrange("p w c -> p (w c)"),
                    lg[:, wc0:wc1, :].rearrange("p w c -> p (w c)"), AFT.Exp)
                nc.vector.reduce_sum(rs[:, wc0:wc1], ex[:, wc0:wc1, :],
                                     axis=mybir.AxisListType.X)
                nc.vector.reciprocal(ri[:, wc0:wc1], rs[:, wc0:wc1])
                for w in range(wc0, wc1):
                    nc.vector.tensor_scalar(outst[:, w, :], ex[:, w, :],
                                            ri[:, w:w + 1], None, op0=AX.mult)
                nc.sync.dma_start(out_d[:, wc0:wc1, :], outst[:, wc0:wc1, :])

    nc.compile()
    return nc


# ──────────────────────────────────────────────────────────────────────
# NTFF profiling shim (only needed when TRACE)
# ──────────────────────────────────────────────────────────────────────

def _install_hook():
    if "antenv.axon_hooks" in sys.modules:
        return
    so_path = "/opt/axon/libaxon_pjrt.so"
    holder = {"hook": None}
    mod = types.ModuleType("antenv.axon_hooks")
    mod.set_axon_ntff_profile_hook = lambda h: holder.__setitem__("hook", h)
    mod.get_axon_ntff_profile_hook = lambda: holder["hook"]
    sys.modules["antenv.axon_hooks"] = mod
    try:
        import antenv
        antenv.axon_hooks = mod
    except ImportError:
        pass
    try:
        lib = ctypes.CDLL(so_path)
        lib.axon_start_nrt_profile.argtypes = [ctypes.POINTER(ctypes.c_int64),
                                               ctypes.c_size_t]
        lib.axon_start_nrt_profile.restype = ctypes.c_int64
        lib.axon_stop_nrt_profile.argtypes = [ctypes.c_char_p]
        lib.axon_stop_nrt_profile.restype = ctypes.c_int64

        @contextlib.contextmanager
        def _hook(output_dir, device_ids):
            import jax
            jax.devices()
            if device_ids:
                ids = (ctypes.c_int64 * len(device_ids))(*device_ids)
                rc = lib.axon_start_nrt_profile(ids, len(device_ids))
            else:
                rc = lib.axon_start_nrt_profile(None, 0)
            if rc != 0:
                raise RuntimeError(f"axon_start_nrt_profile rc={rc}")
            try:
                yield
            finally:
                nf = lib.axon_stop_nrt_profile(str(output_dir).encode())
                if nf < 0:
                    raise RuntimeError(f"axon_stop_nrt_profile rc={nf}")

        holder["hook"] = _hook
    except OSError:
        pass


# ──────────────────────────────────────────────────────────────────────
# entry point
# ──────────────────────────────────────────────────────────────────────

def kernel(x, edge_index, lin_w, conv1_w, conv1_b, conv2_w, conv2_b,
           mlp1_w, mlp1_b, bn_gamma, bn_beta, mlp2_w, mlp2_b):
    x = np.asarray(x, np.float32)
    n, g = x.shape
    D = int(np.asarray(lin_w).shape[1])
    C = int(np.asarray(mlp2_w).shape[1])
    KENC = _cdiv(g, 256) * 256
    K2E = KENC // 256

    plan = _plan(n, np.asarray(edge_index))
    NP, W, BTOT = plan["NP"], plan["W"], plan["BTOT"]

    key = (n, g, D, C, NP,
           tuple(tuple(b) for b in plan["BLK"]), tuple(plan["GB"]))
    if key not in _CACHE:
        _CACHE[key] = _build(1.0 / float(n), NP, W, plan["GB"], plan["GW"],
                             plan["BLK"], plan["OFF"], plan["NST"],
                             plan["SBASE"], BTOT, plan["MAXB"],
                             K2E, D, C)
    nc = _CACHE[key]

    shared = {
        "wlin": _pack_k(lin_w, KENC),
        "w1": _pack_k(conv1_w, D),
        "w2": _pack_k(conv2_w, D),
        "wm1": _pack_k(mlp1_w, D),
        "wm2": _pack_k(mlp2_w, D),
        "b1r": np.ascontiguousarray(
            np.broadcast_to(np.asarray(conv1_b, np.float32), (P, D))),
        "b2r": np.ascontiguousarray(
            np.broadcast_to(np.asarray(conv2_b, np.float32), (P, D))),
        "bcr": np.ascontiguousarray(
            np.broadcast_to(np.asarray(mlp2_b, np.float32), (P, C))),
        "gam": np.ascontiguousarray(
            np.asarray(bn_gamma, np.float32).reshape(2, P).T),
        "bet": np.ascontiguousarray(
            np.asarray(bn_beta, np.float32).reshape(2, P).T),
        "ident": np.eye(P, dtype=np.float32).astype(ml_dtypes.bfloat16),
        "iota": np.ascontiguousarray(np.broadcast_to(
            np.arange(P, dtype=np.float32), (P, plan["MAXB"], P))).astype(
                ml_dtypes.bfloat16),
    }

    in_maps = []
    for c in range(NCORES):
        xs = x[c * NP:(c + 1) * NP]
        if xs.shape[0] < NP:
            xs = np.vstack([xs, np.zeros((NP - xs.shape[0], g), np.float32)])
        xt = _pack_k(np.ascontiguousarray(xs.T), KENC)
        in_maps.append(dict(shared,
                            xt=xt,
                            gidx=plan["gidx"][c],
                            dl=plan["dl"][c],
                            dinv=plan["dinv"][c],
                            valid=plan["valid"][c]))

    if TRACE:
        _install_hook()
        res = run_bass_kernel_spmd(nc, in_maps, core_ids=list(range(NCORES)),
                                   trace=True, **TRACE_KW)
        LAST["exec_time_ns"] = res.exec_time_ns
        LAST["res"] = res
    else:
        res = run_bass_kernel_spmd(nc, in_maps, core_ids=list(range(NCORES)))

    parts = []
    for c in range(NCORES):
        o = np.asarray(res.results[c]["out"])            # [P, W, C]
        parts.append(np.ascontiguousarray(o.transpose(1, 0, 2)).reshape(NP, C))
    return np.concatenate(parts, axis=0)[:n].astype(np.float32)



# revision 80
# speedup vs baseline: 1.3678x; 1.0781x over previous
"""Trainium2 Bass kernel for CelltypeDeconvolver (GCN message passing).

Runs SPMD on 8 NeuronCores. Nodes are partitioned across cores. Per GCN
layer each core computes h_pre = H @ W for its nodes (scaled by
dinv[src]); the dinv-scaled features are exchanged in three pipelined
AllGather window-groups (each fired as soon as its producer windows
finish, overlapping the next stage), landing in replicated per-group
DRAM tables. Edge source rows are then dma_gathered (software DGE,
8-block chunks rotated over the 4 swdge queues, sized so each chunk's
descriptors fit the 128-slot ring) and segment-reduced on the
TensorEngine with 0/1 fp8 selection matrices generated on-device
(is_equal against an iota tile). Self-loops are folded into the PSUM
accumulation via an identity matmul; the next stage's pre-matmul
(conv2 / decoder mlp1 + BN stats) is interleaved per window so the
BN AllReduce fires immediately when conv2 drains. Epilogue PSUM reads
run on the Activation engine to keep the Vector engine free for mask
generation. Graph structure (edge bucketing, degrees, padding) is
prepared host-side in numpy; all float math happens on-device.
"""

import contextlib
import ctypes
import os
import sys
import types

import numpy as np

for _p in ("/opt/trn_rl_repo",):
    if os.path.isdir(_p) and _p not in sys.path:
        sys.path.append(_p)

import ml_dtypes

import concourse.bass as bass
import concourse.bacc as bacc
import concourse.mybir as mybir
from concourse import library_config
from concourse.tile import TileContext
from concourse.bass_utils import run_bass_kernel_spmd

BF16 = mybir.dt.bfloat16
F32 = mybir.dt.float32
FP8 = mybir.dt.float8e4
I16 = mybir.dt.int16
AX = mybir.AluOpType
AFT = mybir.ActivationFunctionType

NCORES = 8
P = 128
BN_EPS = 1e-5
CB = 8             # gather/S chunk size in 128-slot blocks

TRACE = False
TRACE_KW = {}
LAST = {}
_CACHE = {}


def _pack_k(w, kpad):
    """[K, N] f32 -> [K2, 128, 2, N] bf16 packed (k = k2*256 + r*128 + p)."""
    w = np.asarray(w, np.float32)
    k, n = w.shape
    wp = np.zeros((kpad, n), np.float32)
    wp[:k] = w
    k2 = kpad // 256
    return np.ascontiguousarray(
        wp.reshape(k2, 2, P, n).transpose(0, 2, 1, 3)).astype(ml_dtypes.bfloat16)


def _cdiv(a, b):
    return (a + b - 1) // b


# ──────────────────────────────────────────────────────────────────────
# host-side plan: shard nodes, bucket edges, build index / S arrays
# ──────────────────────────────────────────────────────────────────────

def _plan(n, edge_index):
    NP = _cdiv(_cdiv(n, NCORES), P) * P        # nodes per core (multiple of 128)
    W = NP // P                                 # dst windows per core
    # src window groups: first fires its halo exchange earliest, so keep it
    # small; each group's table must stay int16-indexable (<= 32767 rows).
    g1 = _cdiv(W, 4)
    g3 = _cdiv(W, 4)
    GB = [0, g1, W - g3, W]                     # group bounds
    G = len(GB) - 1
    GW = [GB[i + 1] - GB[i] for i in range(G)]  # group widths
    assert all(NCORES * gw * P <= 32767 for gw in GW)

    src = np.asarray(edge_index[0], np.int64)
    dst = np.asarray(edge_index[1], np.int64)
    deg = np.bincount(dst, minlength=n).astype(np.float32) + 1.0
    dinv = (1.0 / np.sqrt(deg)).astype(np.float32)

    c_arr = dst // NP
    w_arr = (dst % NP) // P
    dl_arr = (dst % P).astype(np.int64)
    # src node -> (window group, row in that group's table)
    # group-g table layout = [(c p w), D] over that group's windows
    cs = src // NP
    ii = src % NP
    ws = ii // P
    psrc = ii % P
    grp = np.searchsorted(np.asarray(GB[1:]), ws, side="right").astype(np.int64)
    gw_arr = np.asarray(GW, np.int64)[grp]
    gb_arr = np.asarray(GB[:-1], np.int64)[grp]
    row = (cs * (P * gw_arr) + psrc * gw_arr + (ws - gb_arr)).astype(np.int64)

    order = np.lexsort((row, w_arr, c_arr, grp))
    c_s, w_s, h_s = c_arr[order], w_arr[order], grp[order]
    row_s, dl_s = row[order], dl_arr[order]

    key = ((h_s * NCORES + c_s) * W + w_s)
    cnt = np.bincount(key, minlength=G * NCORES * W).reshape(G, NCORES, W)
    starts = np.zeros(G * NCORES * W + 1, np.int64)
    np.cumsum(cnt.reshape(-1), out=starts[1:])

    # per-group per-window block counts and stream offsets
    BLK = [np.maximum(_cdiv(cnt[g].max(axis=0), P), 1) for g in range(G)]
    OFF = []
    for g in range(G):
        o = np.zeros(W + 1, np.int64)
        np.cumsum(BLK[g], out=o[1:])
        OFF.append(o)
    NST = [int(OFF[g][-1]) for g in range(G)]   # blocks per stream
    SBASE = [int(sum(NST[:g])) for g in range(G)]
    BTOT = int(sum(NST))
    MAXB = max(int(max(b.max() for b in BLK)), CB)

    gidx_list, dl_list = [], []
    for c in range(NCORES):
        fidx = np.zeros(BTOT * P, np.int16)
        fdl = np.full(BTOT * P, -1, np.int64)
        for h in range(G):
            for w in range(W):
                k = (h * NCORES + c) * W + w
                s0, s1 = starts[k], starts[k + 1]
                m = s1 - s0
                if m == 0:
                    continue
                base = (SBASE[h] + OFF[h][w]) * P
                fidx[base:base + m] = row_s[s0:s1].astype(np.int16)
                fdl[base:base + m] = dl_s[s0:s1]
        gidx_list.append(np.ascontiguousarray(
            np.tile(fidx.reshape(-1, 16).T, (NCORES, 1))))
        dl = np.full((P, BTOT, 1), -1.0, np.float32)
        pos = np.nonzero(fdl >= 0)[0]
        dl[pos % P, pos // P, 0] = fdl[pos]
        dl_list.append(dl.astype(ml_dtypes.bfloat16))

    dinv_t, valid_t = [], []
    for c in range(NCORES):
        g = c * NP + (np.arange(P)[:, None] + P * np.arange(W)[None, :])
        real = g < n
        dv = np.zeros((P, W), np.float32)
        dv[real] = dinv[g[real]]
        dinv_t.append(dv)
        valid_t.append(real.astype(np.float32))

    return dict(n=n, NP=NP, W=W, GB=GB, GW=GW,
                BLK=[[int(v) for v in b] for b in BLK],
                OFF=[[int(v) for v in o] for o in OFF],
                NST=NST, SBASE=SBASE, BTOT=BTOT, MAXB=MAXB,
                gidx=gidx_list, dl=dl_list,
                dinv=dinv_t, valid=valid_t)


# ──────────────────────────────────────────────────────────────────────
# device program
# ──────────────────────────────────────────────────────────────────────

def _build(ninv, NP, W, GB, GW, BLK, OFF, NST, SBASE, BTOT,
           MAXB, K2E, D, C):
    RG = [list(range(NCORES))]
    G = len(GW)
    nc = bacc.Bacc("TRN2", num_devices=NCORES, num_swdge_queues=4)

    xt_d = nc.dram_tensor("xt", [K2E, P, 2, NP], BF16, kind="ExternalInput")
    wlin_d = nc.dram_tensor("wlin", [K2E, P, 2, D], BF16, kind="ExternalInput")
    w1_d = nc.dram_tensor("w1", [1, P, 2, D], BF16, kind="ExternalInput")
    w2_d = nc.dram_tensor("w2", [1, P, 2, D], BF16, kind="ExternalInput")
    wm1_d = nc.dram_tensor("wm1", [1, P, 2, D], BF16, kind="ExternalInput")
    wm2_d = nc.dram_tensor("wm2", [1, P, 2, C], BF16, kind="ExternalInput")
    b1r_d = nc.dram_tensor("b1r", [P, D], F32, kind="ExternalInput")
    b2r_d = nc.dram_tensor("b2r", [P, D], F32, kind="ExternalInput")
    bcr_d = nc.dram_tensor("bcr", [P, C], F32, kind="ExternalInput")
    gam_d = nc.dram_tensor("gam", [P, 2], F32, kind="ExternalInput")
    bet_d = nc.dram_tensor("bet", [P, 2], F32, kind="ExternalInput")
    ident_d = nc.dram_tensor("ident", [P, P], BF16, kind="ExternalInput")
    gidx_d = nc.dram_tensor("gidx", [P, BTOT * 8], I16, kind="ExternalInput")
    dl_d = nc.dram_tensor("dl", [P, BTOT, 1], BF16, kind="ExternalInput")
    iota_d = nc.dram_tensor("iota", [P, MAXB, P], BF16, kind="ExternalInput")
    dinv_d = nc.dram_tensor("dinv", [P, W], F32, kind="ExternalInput")
    valid_d = nc.dram_tensor("valid", [P, W], F32, kind="ExternalInput")
    out_d = nc.dram_tensor("out", [P, W, C], F32, kind="ExternalOutput")

    ag_ins = [[nc.dram_tensor(f"ag_in{g}_{i}", [P, GW[g] * D], BF16)
               for g in range(G)] for i in range(2)]
    tables = [[nc.dram_tensor(f"table{g}_{i}", [NCORES * GW[g] * P, D], BF16,
                              addr_space="Shared") for g in range(G)]
              for i in range(2)]
    bn_in = nc.dram_tensor("bn_in", [P, 4], F32)
    bn_out = nc.dram_tensor("bn_out", [P, 4], F32, addr_space="Shared")

    CH = 7                                     # encoder windows per x-chunk

    with TileContext(nc) as tc, contextlib.ExitStack() as ctx:
        cp = ctx.enter_context(tc.tile_pool(name="const", bufs=1))
        big = ctx.enter_context(tc.tile_pool(name="big", bufs=2))
        htp = ctx.enter_context(tc.tile_pool(name="htp", bufs=2))

        nc.gpsimd.load_library(library_config.mlp)

        def cload(dram, shape, dtype, tag, src=None):
            t = cp.tile(shape, dtype, tag=tag, name=tag)
            nc.sync.dma_start(t[:], dram[:] if src is None else src)
            return t

        ident_t = cload(ident_d, [P, P], BF16, "ident")
        w1_t = cload(w1_d, [P, 2, D], BF16, "w1", src=w1_d[0])
        w2_t = cload(w2_d, [P, 2, D], BF16, "w2", src=w2_d[0])
        wm1_t = cload(wm1_d, [P, 2, D], BF16, "wm1", src=wm1_d[0])
        wm2_t = cload(wm2_d, [P, 2, C], BF16, "wm2", src=wm2_d[0])
        b1r_t = cload(b1r_d, [P, D], F32, "b1r")
        b2r_t = cload(b2r_d, [P, D], F32, "b2r")
        bcr_t = cload(bcr_d, [P, C], F32, "bcr")
        gam_t = cload(gam_d, [P, 2], F32, "gam")
        bet_t = cload(bet_d, [P, 2], F32, "bet")
        gidx_t = cload(gidx_d, [P, BTOT * 8], I16, "gidx")
        dl_t = cload(dl_d, [P, BTOT, 1], BF16, "dl")
        iota_t = cload(iota_d, [P, MAXB, P], BF16, "iota")
        dinv_t = cload(dinv_d, [P, W], F32, "dinv")
        valid_t = cload(valid_d, [P, W], F32, "valid")

        # persistent activations: ht slots rotate h0T -> h1T -> h2T -> h4T
        ht = [htp.tile([P, 2, NP], BF16, tag="ht", name=f"ht{i}")
              for i in range(3)]
        # hpre0 / hpre1 / h3 share one 2-deep rotation: h3 (layer-2 output)
        # reuses hpre0's buffer, whose last reader is layer 1's self-loop.
        hpre = [big.tile([P, W, D], BF16, tag="bigbuf", name=f"hpre{i}")
                for i in range(2)]
        h3 = big.tile([P, 2, NP], BF16, tag="bigbuf", name="h3")
        sumps = cp.tile([P, 2, W], F32, tag="sumps")
        sqps = cp.tile([P, 2, W], F32, tag="sqps")

        def send(li, g):
            nc.sync.dma_start(ag_ins[li][g][:],
                              hpre[li][:, GB[g]:GB[g + 1], :])
            nc.gpsimd.collective_compute(
                "AllGather", AX.bypass, ins=[ag_ins[li][g][:]],
                outs=[tables[li][g][:]], replica_groups=RG)

        send_at = {GB[g + 1] - 1: g for g in range(G)}

        # ── encoder: h0 = x @ lin_w (node-major) → transpose → ht[0],
        #    with conv1's pre-matmul interleaved per window
        with tc.tile_pool(name="encw", bufs=2) as wp, \
             tc.tile_pool(name="encp", bufs=2, space="PSUM") as pp, \
             tc.tile_pool(name="xtp", bufs=2) as xtp:
            wlin_t = []
            for k2 in range(K2E):
                t = cp.tile([P, 2, D], BF16, tag=f"wlin{k2}", name=f"wlin{k2}")
                nc.sync.dma_start(t[:], wlin_d[k2])
                wlin_t.append(t)
            for wc in range(_cdiv(W, CH)):
                ws, we = wc * CH, min(W, (wc + 1) * CH)
                xtc = []
                for k2 in range(K2E):
                    t = xtp.tile([P, 2, CH * P], BF16, tag=f"xtc{k2}",
                                 name=f"xtc{k2}_{wc}")
                    eng = nc.sync if k2 % 2 == 0 else nc.scalar
                    eng.dma_start(t[:, :, :(we - ws) * P],
                                  xt_d[k2][:, :, ws * P:we * P])
                    xtc.append(t)
                for w in range(ws, we):
                    lsl = slice((w - ws) * P, (w - ws + 1) * P)
                    sl = slice(w * P, (w + 1) * P)
                    ps = pp.tile([P, D], F32, tag="ps", name=f"eps{w}")
                    for k2 in range(K2E):
                        for r in range(2):
                            nc.tensor.matmul(
                                ps[:], xtc[k2][:, r, lsl], wlin_t[k2][:, r, :],
                                start=(k2 == 0 and r == 0),
                                stop=(k2 == K2E - 1 and r == 1))
                    hb = wp.tile([P, D], BF16, tag="hb", name=f"ehb{w}")
                    nc.vector.tensor_copy(hb[:], ps[:])
                    for r in range(2):
                        pt = pp.tile([P, P], BF16, tag="pt", name=f"ept{w}_{r}")
                        nc.tensor.transpose(pt[:], hb[:, r * P:(r + 1) * P],
                                            ident_t[:])
                        nc.vector.tensor_copy(ht[0][:, r, sl], pt[:])
                    ps2 = pp.tile([P, D], F32, tag="ps_pre", name=f"pre0_{w}")
                    for r in range(2):
                        nc.tensor.matmul(ps2[:], ht[0][:, r, sl], w1_t[:, r, :],
                                         start=(r == 0), stop=(r == 1))
                    nc.scalar.activation(hpre[0][:, w, :], ps2[:], AFT.Copy,
                                         scale=dinv_t[:, w:w + 1])
                    if w in send_at:
                        send(0, send_at[w])

        # ── conv layers (layer li consumes tableA/B[li]; the next stage's
        #    pre-matmul + halo send are interleaved into this layer's loop)
        for li in range(2):
            HT_out = ht[li + 1]
            br = b1r_t if li == 0 else b2r_t
            with tc.tile_pool(name=f"cw{li}", bufs=3) as wp, \
                 tc.tile_pool(name=f"cp{li}", bufs=2, space="PSUM") as pp:

                chunks = {}
                qc = [0]

                def _get_chunk(hs, ci, chunks=chunks, wp=wp, li=li):
                    key = (hs, ci)
                    if key in chunks:
                        return chunks[key]
                    nstream = NST[hs]
                    base_blk = SBASE[hs] + ci * CB
                    nblk = min(CB, nstream - ci * CB)
                    nn = nblk * P
                    gt = wp.tile([P, CB, D], BF16, tag=f"gt{hs}",
                                 name=f"gt{li}_{hs}_{ci}", bufs=4)
                    stt = wp.tile([P, CB, P], FP8, tag=f"st{hs}",
                                  name=f"st{li}_{hs}_{ci}", bufs=4)
                    tb = tables[li][hs][:]
                    nc.gpsimd.dma_gather(
                        gt[:, :nblk, :], tb,
                        gidx_t[:, base_blk * 8:(base_blk + nblk) * 8],
                        nn, nn, D, single_packet=True,
                        queue_num=qc[0] % 4)
                    qc[0] += 1
                    # build the 0/1 selection block on-device: S[e,b,j] =
                    # (j == dst_lane[e,b]); padding slots have dl = -1.
                    in0, in1 = bass.broadcast_tensor_aps(
                        iota_t[:, :nblk, :],
                        dl_t[:, base_blk:base_blk + nblk, :])
                    nc.vector.tensor_tensor(stt[:, :nblk, :], in0, in1,
                                            op=AX.is_equal)
                    chunks[key] = (gt, stt)
                    return chunks[key]

                def chunk_spans(off, nblk):
                    out = []
                    b = off
                    while b < off + nblk:
                        ci = b // CB
                        b1 = min(off + nblk, (ci + 1) * CB)
                        out.append((ci, b - ci * CB, b1 - ci * CB))
                        b = b1
                    return out

                for w in range(W):
                    sl = slice(w * P, (w + 1) * P)
                    pa = pp.tile([P, D], F32, tag="ps_agg", name=f"agg{li}_{w}")
                    spans = [(g, s) for g in range(G)
                             for s in chunk_spans(OFF[g][w], BLK[g][w])]
                    nmm = sum(s[2] - s[1] for _, s in spans)
                    # self-loop folded into psum: pa = hpre[w] + sum S.gt
                    nc.tensor.matmul(pa[:], ident_t[:], hpre[li][:, w, :],
                                     start=True, stop=False)
                    mi = 0
                    for hs, (ci, b0, b1) in spans:
                        gt, stt = _get_chunk(hs, ci)
                        for b in range(b0, b1):
                            nc.tensor.matmul(pa[:], stt[:, b, :], gt[:, b, :],
                                             start=False,
                                             stop=(mi == nmm - 1))
                            mi += 1
                    tf2 = wp.tile([P, D], F32, tag="tf2", name=f"tf2{li}_{w}")
                    nc.vector.scalar_tensor_tensor(
                        tf2[:], pa[:], dinv_t[:, w:w + 1], br[:],
                        op0=AX.mult, op1=AX.add)
                    hb = wp.tile([P, D], BF16, tag="hb2", name=f"chb{li}_{w}")
                    nc.scalar.activation(hb[:], tf2[:], AFT.Relu,
                                         scale=valid_t[:, w:w + 1])
                    for r in range(2):
                        pt = pp.tile([P, P], BF16, tag="pt",
                                     name=f"cpt{li}_{w}_{r}")
                        nc.tensor.transpose(pt[:], hb[:, r * P:(r + 1) * P],
                                            ident_t[:])
                        nc.vector.tensor_copy(HT_out[:, r, sl], pt[:])
                    if li == 0:
                        ps2 = pp.tile([P, D], F32, tag="ps_pre",
                                      name=f"pre1_{w}")
                        for r in range(2):
                            nc.tensor.matmul(ps2[:], HT_out[:, r, sl],
                                             w2_t[:, r, :],
                                             start=(r == 0), stop=(r == 1))
                        nc.scalar.activation(hpre[1][:, w, :], ps2[:], AFT.Copy,
                                             scale=dinv_t[:, w:w + 1])
                        if w in send_at:
                            send(1, send_at[w])
                    else:
                        # decoder mlp1 per window (feat-major) + BN stats
                        for fb in range(2):
                            pm = pp.tile([P, P], F32, tag=f"pm{fb}",
                                         name=f"pm{fb}_{w}")
                            for r in range(2):
                                nc.tensor.matmul(
                                    pm[:], wm1_t[:, r, fb * P:(fb + 1) * P],
                                    HT_out[:, r, sl],
                                    start=(r == 0), stop=(r == 1))
                            nc.vector.tensor_scalar(
                                h3[:, fb, sl], pm[:], 1.0, 0.0, op0=AX.mult,
                                op1=AX.add, accum_out=sumps[:, fb, w:w + 1])
                            scr = wp.tile([P, P], F32, tag=f"scr{fb}",
                                          name=f"scr{fb}_{w}")
                            nc.vector.scalar_tensor_tensor(
                                scr[:], h3[:, fb, sl], 1.0, h3[:, fb, sl],
                                op0=AX.mult, op1=AX.mult,
                                accum_out=sqps[:, fb, w:w + 1])

        # ── decoder: BN + relu + mlp2 + softmax (mlp1 ran inside layer 2)
        ht4 = htp.tile([P, 2, NP], BF16, tag="ht", name="ht4")
        with tc.tile_pool(name="dec", bufs=2) as wp, \
             tc.tile_pool(name="decp", bufs=2, space="PSUM") as pp, \
             tc.tile_pool(name="st1", bufs=1) as sp:
            sums = sp.tile([P, 2], F32, tag="sums")
            sqs = sp.tile([P, 2], F32, tag="sqs")
            for fb in range(2):
                nc.vector.reduce_sum(sums[:, fb:fb + 1], sumps[:, fb, :],
                                     axis=mybir.AxisListType.X)
                nc.vector.reduce_sum(sqs[:, fb:fb + 1], sqps[:, fb, :],
                                     axis=mybir.AxisListType.X)
            bnio = sp.tile([P, 4], F32, tag="bnio")
            nc.vector.tensor_copy(bnio[:, 0:2], sums[:])
            nc.vector.tensor_copy(bnio[:, 2:4], sqs[:])
            nc.sync.dma_start(bn_in[:], bnio[:])
            nc.gpsimd.collective_compute(
                "AllReduce", AX.add, ins=[bn_in[:]], outs=[bn_out[:]],
                replica_groups=RG)
            bns = sp.tile([P, 4], F32, tag="bns")
            nc.sync.dma_start(bns[:], bn_out[:])

            mu = sp.tile([P, 2], F32, tag="mu")
            nc.vector.tensor_scalar(mu[:], bns[:, 0:2], ninv, None, op0=AX.mult)
            msq = sp.tile([P, 2], F32, tag="msq")
            nc.vector.tensor_tensor(msq[:], mu[:], mu[:], op=AX.mult)
            var = sp.tile([P, 2], F32, tag="var")
            nc.vector.scalar_tensor_tensor(var[:], bns[:, 2:4], ninv, msq[:],
                                           op0=AX.mult, op1=AX.subtract)
            vae = sp.tile([P, 2], F32, tag="vae")
            nc.vector.tensor_scalar(vae[:], var[:], BN_EPS, None, op0=AX.add)
            sd = sp.tile([P, 2], F32, tag="sd")
            nc.scalar.activation(sd[:], vae[:], AFT.Sqrt)
            rstd = sp.tile([P, 2], F32, tag="rstd")
            nc.vector.reciprocal(rstd[:], sd[:])
            A = sp.tile([P, 2], F32, tag="A")
            nc.vector.tensor_tensor(A[:], rstd[:], gam_t[:], op=AX.mult)
            tb = sp.tile([P, 2], F32, tag="tb")
            nc.vector.tensor_tensor(tb[:], mu[:], A[:], op=AX.mult)
            B = sp.tile([P, 2], F32, tag="B")
            nc.vector.tensor_tensor(B[:], bet_t[:], tb[:], op=AX.subtract)

            lg = sp.tile([P, W, C], F32, tag="lg")
            ex = sp.tile([P, W, C], F32, tag="ex")
            rs = sp.tile([P, W], F32, tag="rs")
            ri = sp.tile([P, W], F32, tag="ri")
            outst = sp.tile([P, W, C], F32, tag="outst")
            TW = 12
            for wc0 in range(0, W, TW):
                wc1 = min(W, wc0 + TW)
                csl = slice(wc0 * P, wc1 * P)
                for fb in range(2):
                    nc.scalar.activation(ht4[:, fb, csl], h3[:, fb, csl],
                                         AFT.Relu, bias=B[:, fb:fb + 1],
                                         scale=A[:, fb:fb + 1])
                for w in range(wc0, wc1):
                    sl = slice(w * P, (w + 1) * P)
                    pl = pp.tile([P, C], F32, tag="ps_lg", name=f"plg{w}")
                    for r in range(2):
                        nc.tensor.matmul(pl[:], ht4[:, r, sl], wm2_t[:, r, :],
                                         start=(r == 0), stop=(r == 1))
                    nc.vector.scalar_tensor_tensor(lg[:, w, :], pl[:], 1.0,
                                                   bcr_t[:],
                                                   op0=AX.mult, op1=AX.add)
                nc.scalar.activation(
                    ex[:, wc0:wc1, :].rearrange("p w c -> p (w c)"),
                    lg[:, wc0:wc1, :].rearrange("p w c -> p (w c)"), AFT.Exp)
                nc.vector.reduce_sum(rs[:, wc0:wc1], ex[:, wc0:wc1, :],
                                     axis=mybir.AxisListType.X)
                nc.vector.reciprocal(ri[:, wc0:wc1], rs[:, wc0:wc1])
                for w in range(wc0, wc1):
                    nc.vector.tensor_scalar(outst[:, w, :], ex[:, w, :],
                                            ri[:, w:w + 1], None, op0=AX.mult)
                nc.sync.dma_start(out_d[:, wc0:wc1, :], outst[:, wc0:wc1, :])

    nc.compile()
    return nc


# ──────────────────────────────────────────────────────────────────────
# NTFF profiling shim (only needed when TRACE)
# ──────────────────────────────────────────────────────────────────────

def _install_hook():
    if "antenv.axon_hooks" in sys.modules:
        return
    so_path = "/opt/axon/libaxon_pjrt.so"
    holder = {"hook": None}
    mod = types.ModuleType("antenv.axon_hooks")
    mod.set_axon_ntff_profile_hook = lambda h: holder.__setitem__("hook", h)
    mod.get_axon_ntff_profile_hook = lambda: holder["hook"]
    sys.modules["antenv.axon_hooks"] = mod
    try:
        import antenv
        antenv.axon_hooks = mod
    except ImportError:
        pass
    try:
        lib = ctypes.CDLL(so_path)
        lib.axon_start_nrt_profile.argtypes = [ctypes.POINTER(ctypes.c_int64),
                                               ctypes.c_size_t]
        lib.axon_start_nrt_profile.restype = ctypes.c_int64
        lib.axon_stop_nrt_profile.argtypes = [ctypes.c_char_p]
        lib.axon_stop_nrt_profile.restype = ctypes.c_int64

        @contextlib.contextmanager
        def _hook(output_dir, device_ids):
            import jax
            jax.devices()
            if device_ids:
                ids = (ctypes.c_int64 * len(device_ids))(*device_ids)
                rc = lib.axon_start_nrt_profile(ids, len(device_ids))
            else:
                rc = lib.axon_start_nrt_profile(None, 0)
            if rc != 0:
                raise RuntimeError(f"axon_start_nrt_profile rc={rc}")
            try:
                yield
            finally:
                nf = lib.axon_stop_nrt_profile(str(output_dir).encode())
                if nf < 0:
                    raise RuntimeError(f"axon_stop_nrt_profile rc={nf}")

        holder["hook"] = _hook
    except OSError:
        pass


# ──────────────────────────────────────────────────────────────────────
# entry point
# ──────────────────────────────────────────────────────────────────────

def kernel(x, edge_index, lin_w, conv1_w, conv1_b, conv2_w, conv2_b,
           mlp1_w, mlp1_b, bn_gamma, bn_beta, mlp2_w, mlp2_b):
    x = np.asarray(x, np.float32)
    n, g = x.shape
    D = int(np.asarray(lin_w).shape[1])
    C = int(np.asarray(mlp2_w).shape[1])
    KENC = _cdiv(g, 256) * 256
    K2E = KENC // 256

    plan = _plan(n, np.asarray(edge_index))
    NP, W, BTOT = plan["NP"], plan["W"], plan["BTOT"]

    key = (n, g, D, C, NP,
           tuple(tuple(b) for b in plan["BLK"]), tuple(plan["GB"]))
    if key not in _CACHE:
        _CACHE[key] = _build(1.0 / float(n), NP, W, plan["GB"], plan["GW"],
                             plan["BLK"], plan["OFF"], plan["NST"],
                             plan["SBASE"], BTOT, plan["MAXB"],
                             K2E, D, C)
    nc = _CACHE[key]

    shared = {
        "wlin": _pack_k(lin_w, KENC),
        "w1": _pack_k(conv1_w, D),
        "w2": _pack_k(conv2_w, D),
        "wm1": _pack_k(mlp1_w, D),
        "wm2": _pack_k(mlp2_w, D),
        "b1r": np.ascontiguousarray(
            np.broadcast_to(np.asarray(conv1_b, np.float32), (P, D))),
        "b2r": np.ascontiguousarray(
            np.broadcast_to(np.asarray(conv2_b, np.float32), (P, D))),
        "bcr": np.ascontiguousarray(
            np.broadcast_to(np.asarray(mlp2_b, np.float32), (P, C))),
        "gam": np.ascontiguousarray(
            np.asarray(bn_gamma, np.float32).reshape(2, P).T),
        "bet": np.ascontiguousarray(
            np.asarray(bn_beta, np.float32).reshape(2, P).T),
        "ident": np.eye(P, dtype=np.float32).astype(ml_dtypes.bfloat16),
        "iota": np.ascontiguousarray(np.broadcast_to(
            np.arange(P, dtype=np.float32), (P, plan["MAXB"], P))).astype(
                ml_dtypes.bfloat16),
    }

    in_maps = []
    for c in range(NCORES):
        xs = x[c * NP:(c + 1) * NP]
        if xs.shape[0] < NP:
            xs = np.vstack([xs, np.zeros((NP - xs.shape[0], g), np.float32)])
        xt = _pack_k(np.ascontiguousarray(xs.T), KENC)
        in_maps.append(dict(shared,
                            xt=xt,
                            gidx=plan["gidx"][c],
                            dl=plan["dl"][c],
                            dinv=plan["dinv"][c],
                            valid=plan["valid"][c]))

    if TRACE:
        _install_hook()
        res = run_bass_kernel_spmd(nc, in_maps, core_ids=list(range(NCORES)),
                                   trace=True, **TRACE_KW)
        LAST["exec_time_ns"] = res.exec_time_ns
        LAST["res"] = res
    else:
        res = run_bass_kernel_spmd(nc, in_maps, core_ids=list(range(NCORES)))

    parts = []
    for c in range(NCORES):
        o = np.asarray(res.results[c]["out"])            # [P, W, C]
        parts.append(np.ascontiguousarray(o.transpose(1, 0, 2)).reshape(NP, C))
    return np.concatenate(parts, axis=0)[:n].astype(np.float32)



# revision 82
# speedup vs baseline: 1.3723x; 1.0033x over previous
"""Trainium2 Bass kernel for CelltypeDeconvolver (GCN message passing).

Runs SPMD on 8 NeuronCores. Nodes are partitioned across cores. Per GCN
layer each core computes h_pre = H @ W for its nodes (scaled by
dinv[src]); the dinv-scaled features are exchanged in three pipelined
AllGather window-groups (each fired as soon as its producer windows
finish, overlapping the next stage), landing in replicated per-group
DRAM tables. Edge source rows are then dma_gathered (software DGE,
8-block chunks rotated over the 4 swdge queues, sized so each chunk's
descriptors fit the 128-slot ring) and segment-reduced on the
TensorEngine with 0/1 fp8 selection matrices generated on-device
(is_equal against an iota tile). Self-loops are folded into the PSUM
accumulation via an identity matmul; the next stage's pre-matmul
(conv2 / decoder mlp1 + BN stats) is interleaved per window so the
BN AllReduce fires immediately when conv2 drains. Epilogue PSUM reads
run on the Activation engine to keep the Vector engine free for mask
generation. Graph structure (edge bucketing, degrees, padding) is
prepared host-side in numpy; all float math happens on-device.
"""

import contextlib
import ctypes
import os
import sys
import types

import numpy as np

for _p in ("/opt/trn_rl_repo",):
    if os.path.isdir(_p) and _p not in sys.path:
        sys.path.append(_p)

import ml_dtypes

import concourse.bass as bass
import concourse.bacc as bacc
import concourse.mybir as mybir
from concourse import library_config
from concourse.tile import TileContext
from concourse.bass_utils import run_bass_kernel_spmd

BF16 = mybir.dt.bfloat16
F32 = mybir.dt.float32
FP8 = mybir.dt.float8e4
I16 = mybir.dt.int16
AX = mybir.AluOpType
AFT = mybir.ActivationFunctionType

NCORES = 8
P = 128
BN_EPS = 1e-5
CB = 8             # gather/S chunk size in 128-slot blocks

TRACE = False
TRACE_KW = {}
LAST = {}
_CACHE = {}


def _pack_k(w, kpad):
    """[K, N] f32 -> [K2, 128, 2, N] bf16 packed (k = k2*256 + r*128 + p)."""
    w = np.asarray(w, np.float32)
    k, n = w.shape
    wp = np.zeros((kpad, n), np.float32)
    wp[:k] = w
    k2 = kpad // 256
    return np.ascontiguousarray(
        wp.reshape(k2, 2, P, n).transpose(0, 2, 1, 3)).astype(ml_dtypes.bfloat16)


def _cdiv(a, b):
    return (a + b - 1) // b


# ──────────────────────────────────────────────────────────────────────
# host-side plan: shard nodes, bucket edges, build index / S arrays
# ──────────────────────────────────────────────────────────────────────

def _plan(n, edge_index):
    NP = _cdiv(_cdiv(n, NCORES), P) * P        # nodes per core (multiple of 128)
    W = NP // P                                 # dst windows per core
    # src window groups: first fires its halo exchange earliest, so keep it
    # small; each group's table must stay int16-indexable (<= 32767 rows).
    g1 = _cdiv(W, 4)
    g3 = _cdiv(W, 4)
    GB = [0, g1, W - g3, W]                     # group bounds
    G = len(GB) - 1
    GW = [GB[i + 1] - GB[i] for i in range(G)]  # group widths
    assert all(NCORES * gw * P <= 32767 for gw in GW)

    src = np.asarray(edge_index[0], np.int64)
    dst = np.asarray(edge_index[1], np.int64)
    deg = np.bincount(dst, minlength=n).astype(np.float32) + 1.0
    dinv = (1.0 / np.sqrt(deg)).astype(np.float32)

    c_arr = dst // NP
    w_arr = (dst % NP) // P
    dl_arr = (dst % P).astype(np.int64)
    # src node -> (window group, row in that group's table)
    # group-g table layout = [(c p w), D] over that group's windows
    cs = src // NP
    ii = src % NP
    ws = ii // P
    psrc = ii % P
    grp = np.searchsorted(np.asarray(GB[1:]), ws, side="right").astype(np.int64)
    gw_arr = np.asarray(GW, np.int64)[grp]
    gb_arr = np.asarray(GB[:-1], np.int64)[grp]
    row = (cs * (P * gw_arr) + psrc * gw_arr + (ws - gb_arr)).astype(np.int64)

    order = np.lexsort((row, w_arr, c_arr, grp))
    c_s, w_s, h_s = c_arr[order], w_arr[order], grp[order]
    row_s, dl_s = row[order], dl_arr[order]

    key = ((h_s * NCORES + c_s) * W + w_s)
    cnt = np.bincount(key, minlength=G * NCORES * W).reshape(G, NCORES, W)
    starts = np.zeros(G * NCORES * W + 1, np.int64)
    np.cumsum(cnt.reshape(-1), out=starts[1:])

    # per-group per-window block counts and stream offsets
    BLK = [np.maximum(_cdiv(cnt[g].max(axis=0), P), 1) for g in range(G)]
    OFF = []
    for g in range(G):
        o = np.zeros(W + 1, np.int64)
        np.cumsum(BLK[g], out=o[1:])
        OFF.append(o)
    NST = [int(OFF[g][-1]) for g in range(G)]   # blocks per stream
    SBASE = [int(sum(NST[:g])) for g in range(G)]
    BTOT = int(sum(NST))
    MAXB = max(int(max(b.max() for b in BLK)), CB)

    gidx_list, dl_list = [], []
    for c in range(NCORES):
        fidx = np.zeros(BTOT * P, np.int16)
        fdl = np.full(BTOT * P, -1, np.int64)
        for h in range(G):
            for w in range(W):
                k = (h * NCORES + c) * W + w
                s0, s1 = starts[k], starts[k + 1]
                m = s1 - s0
                if m == 0:
                    continue
                base = (SBASE[h] + OFF[h][w]) * P
                fidx[base:base + m] = row_s[s0:s1].astype(np.int16)
                fdl[base:base + m] = dl_s[s0:s1]
        gidx_list.append(np.ascontiguousarray(
            np.tile(fidx.reshape(-1, 16).T, (NCORES, 1))))
        dl = np.full((P, BTOT, 1), -1.0, np.float32)
        pos = np.nonzero(fdl >= 0)[0]
        dl[pos % P, pos // P, 0] = fdl[pos]
        dl_list.append(dl.astype(ml_dtypes.bfloat16))

    dinv_t, valid_t = [], []
    for c in range(NCORES):
        g = c * NP + (np.arange(P)[:, None] + P * np.arange(W)[None, :])
        real = g < n
        dv = np.zeros((P, W), np.float32)
        dv[real] = dinv[g[real]]
        dinv_t.append(dv)
        valid_t.append(real.astype(np.float32))

    return dict(n=n, NP=NP, W=W, GB=GB, GW=GW,
                BLK=[[int(v) for v in b] for b in BLK],
                OFF=[[int(v) for v in o] for o in OFF],
                NST=NST, SBASE=SBASE, BTOT=BTOT, MAXB=MAXB,
                gidx=gidx_list, dl=dl_list,
                dinv=dinv_t, valid=valid_t)


# ──────────────────────────────────────────────────────────────────────
# device program
# ──────────────────────────────────────────────────────────────────────

def _build(ninv, NP, W, GB, GW, BLK, OFF, NST, SBASE, BTOT,
           MAXB, K2E, D, C):
    RG = [list(range(NCORES))]
    G = len(GW)
    nc = bacc.Bacc("TRN2", num_devices=NCORES, num_swdge_queues=4)

    xt_d = nc.dram_tensor("xt", [K2E, P, 2, NP], BF16, kind="ExternalInput")
    wlin_d = nc.dram_tensor("wlin", [K2E, P, 2, D], BF16, kind="ExternalInput")
    w1_d = nc.dram_tensor("w1", [1, P, 2, D], BF16, kind="ExternalInput")
    w2_d = nc.dram_tensor("w2", [1, P, 2, D], BF16, kind="ExternalInput")
    wm1_d = nc.dram_tensor("wm1", [1, P, 2, D], BF16, kind="ExternalInput")
    wm2_d = nc.dram_tensor("wm2", [1, P, 2, C], BF16, kind="ExternalInput")
    b1r_d = nc.dram_tensor("b1r", [P, D], F32, kind="ExternalInput")
    b2r_d = nc.dram_tensor("b2r", [P, D], F32, kind="ExternalInput")
    bcr_d = nc.dram_tensor("bcr", [P, C], F32, kind="ExternalInput")
    gam_d = nc.dram_tensor("gam", [P, 2], F32, kind="ExternalInput")
    bet_d = nc.dram_tensor("bet", [P, 2], F32, kind="ExternalInput")
    ident_d = nc.dram_tensor("ident", [P, P], BF16, kind="ExternalInput")
    gidx_d = nc.dram_tensor("gidx", [P, BTOT * 8], I16, kind="ExternalInput")
    dl_d = nc.dram_tensor("dl", [P, BTOT, 1], BF16, kind="ExternalInput")
    iota_d = nc.dram_tensor("iota", [P, MAXB, P], BF16, kind="ExternalInput")
    dinv_d = nc.dram_tensor("dinv", [P, W], F32, kind="ExternalInput")
    valid_d = nc.dram_tensor("valid", [P, W], F32, kind="ExternalInput")
    out_d = nc.dram_tensor("out", [P, W, C], F32, kind="ExternalOutput")

    ag_ins = [[nc.dram_tensor(f"ag_in{g}_{i}", [P, GW[g] * D], BF16)
               for g in range(G)] for i in range(2)]
    tables = [[nc.dram_tensor(f"table{g}_{i}", [NCORES * GW[g] * P, D], BF16,
                              addr_space="Shared") for g in range(G)]
              for i in range(2)]
    bn_in = nc.dram_tensor("bn_in", [P, 4], F32)
    bn_out = nc.dram_tensor("bn_out", [P, 4], F32, addr_space="Shared")

    CH = 7                                     # encoder windows per x-chunk

    with TileContext(nc) as tc, contextlib.ExitStack() as ctx:
        cp = ctx.enter_context(tc.tile_pool(name="const", bufs=1))
        big = ctx.enter_context(tc.tile_pool(name="big", bufs=2))
        htp = ctx.enter_context(tc.tile_pool(name="htp", bufs=2))

        nc.gpsimd.load_library(library_config.mlp)

        def cload(dram, shape, dtype, tag, src=None):
            t = cp.tile(shape, dtype, tag=tag, name=tag)
            nc.sync.dma_start(t[:], dram[:] if src is None else src)
            return t

        ident_t = cload(ident_d, [P, P], BF16, "ident")
        w1_t = cload(w1_d, [P, 2, D], BF16, "w1", src=w1_d[0])
        w2_t = cload(w2_d, [P, 2, D], BF16, "w2", src=w2_d[0])
        wm1_t = cload(wm1_d, [P, 2, D], BF16, "wm1", src=wm1_d[0])
        wm2_t = cload(wm2_d, [P, 2, C], BF16, "wm2", src=wm2_d[0])
        b1r_t = cload(b1r_d, [P, D], F32, "b1r")
        b2r_t = cload(b2r_d, [P, D], F32, "b2r")
        bcr_t = cload(bcr_d, [P, C], F32, "bcr")
        gam_t = cload(gam_d, [P, 2], F32, "gam")
        bet_t = cload(bet_d, [P, 2], F32, "bet")
        gidx_t = cload(gidx_d, [P, BTOT * 8], I16, "gidx")
        dl_t = cload(dl_d, [P, BTOT, 1], BF16, "dl")
        iota_t = cload(iota_d, [P, MAXB, P], BF16, "iota")
        dinv_t = cload(dinv_d, [P, W], F32, "dinv")
        valid_t = cload(valid_d, [P, W], F32, "valid")

        # persistent activations: ht slots rotate h0T -> h1T -> h2T -> h4T
        ht = [htp.tile([P, 2, NP], BF16, tag="ht", name=f"ht{i}")
              for i in range(3)]
        # hpre0 / hpre1 / h3 share one 2-deep rotation: h3 (layer-2 output)
        # reuses hpre0's buffer, whose last reader is layer 1's self-loop.
        hpre = [big.tile([P, W, D], BF16, tag="bigbuf", name=f"hpre{i}")
                for i in range(2)]
        h3 = big.tile([P, 2, NP], BF16, tag="bigbuf", name="h3")
        sumps = cp.tile([P, 2, W], F32, tag="sumps")
        sqps = cp.tile([P, 2, W], F32, tag="sqps")

        def send(li, g):
            nc.sync.dma_start(ag_ins[li][g][:],
                              hpre[li][:, GB[g]:GB[g + 1], :])
            nc.gpsimd.collective_compute(
                "AllGather", AX.bypass, ins=[ag_ins[li][g][:]],
                outs=[tables[li][g][:]], replica_groups=RG)

        send_at = {GB[g + 1] - 1: g for g in range(G)}

        # ── encoder: h0 = x @ lin_w (node-major) → transpose → ht[0],
        #    with conv1's pre-matmul interleaved per window
        with tc.tile_pool(name="encw", bufs=2) as wp, \
             tc.tile_pool(name="encp", bufs=2, space="PSUM") as pp, \
             tc.tile_pool(name="xtp", bufs=2) as xtp:
            wlin_t = []
            for k2 in range(K2E):
                t = cp.tile([P, 2, D], BF16, tag=f"wlin{k2}", name=f"wlin{k2}")
                nc.sync.dma_start(t[:], wlin_d[k2])
                wlin_t.append(t)
            for wc in range(_cdiv(W, CH)):
                ws, we = wc * CH, min(W, (wc + 1) * CH)
                xtc = []
                for k2 in range(K2E):
                    t = xtp.tile([P, 2, CH * P], BF16, tag=f"xtc{k2}",
                                 name=f"xtc{k2}_{wc}")
                    eng = nc.sync if k2 % 2 == 0 else nc.scalar
                    eng.dma_start(t[:, :, :(we - ws) * P],
                                  xt_d[k2][:, :, ws * P:we * P])
                    xtc.append(t)
                for w in range(ws, we):
                    lsl = slice((w - ws) * P, (w - ws + 1) * P)
                    sl = slice(w * P, (w + 1) * P)
                    ps = pp.tile([P, D], F32, tag="ps", name=f"eps{w}")
                    for k2 in range(K2E):
                        for r in range(2):
                            nc.tensor.matmul(
                                ps[:], xtc[k2][:, r, lsl], wlin_t[k2][:, r, :],
                                start=(k2 == 0 and r == 0),
                                stop=(k2 == K2E - 1 and r == 1))
                    hb = wp.tile([P, D], BF16, tag="hb", name=f"ehb{w}")
                    nc.vector.tensor_copy(hb[:], ps[:])
                    for r in range(2):
                        pt = pp.tile([P, P], BF16, tag="pt", name=f"ept{w}_{r}")
                        nc.tensor.transpose(pt[:], hb[:, r * P:(r + 1) * P],
                                            ident_t[:])
                        nc.vector.tensor_copy(ht[0][:, r, sl], pt[:])
                    ps2 = pp.tile([P, D], F32, tag="ps_pre", name=f"pre0_{w}")
                    for r in range(2):
                        nc.tensor.matmul(ps2[:], ht[0][:, r, sl], w1_t[:, r, :],
                                         start=(r == 0), stop=(r == 1))
                    nc.scalar.activation(hpre[0][:, w, :], ps2[:], AFT.Copy,
                                         scale=dinv_t[:, w:w + 1])
                    if w in send_at:
                        send(0, send_at[w])

        # ── conv layers (layer li consumes tableA/B[li]; the next stage's
        #    pre-matmul + halo send are interleaved into this layer's loop)
        for li in range(2):
            HT_out = ht[li + 1]
            br = b1r_t if li == 0 else b2r_t
            with tc.tile_pool(name=f"cw{li}", bufs=3) as wp, \
                 tc.tile_pool(name=f"cp{li}", bufs=2, space="PSUM") as pp:

                chunks = {}
                qc = [0]

                def _get_chunk(hs, ci, chunks=chunks, wp=wp, li=li):
                    key = (hs, ci)
                    if key in chunks:
                        return chunks[key]
                    nstream = NST[hs]
                    base_blk = SBASE[hs] + ci * CB
                    nblk = min(CB, nstream - ci * CB)
                    nn = nblk * P
                    gt = wp.tile([P, CB, D], BF16, tag=f"gt{hs}",
                                 name=f"gt{li}_{hs}_{ci}", bufs=5)
                    stt = wp.tile([P, CB, P], FP8, tag=f"st{hs}",
                                  name=f"st{li}_{hs}_{ci}", bufs=5)
                    tb = tables[li][hs][:]
                    nc.gpsimd.dma_gather(
                        gt[:, :nblk, :], tb,
                        gidx_t[:, base_blk * 8:(base_blk + nblk) * 8],
                        nn, nn, D, single_packet=True,
                        queue_num=qc[0] % 4)
                    qc[0] += 1
                    # build the 0/1 selection block on-device: S[e,b,j] =
                    # (j == dst_lane[e,b]); padding slots have dl = -1.
                    in0, in1 = bass.broadcast_tensor_aps(
                        iota_t[:, :nblk, :],
                        dl_t[:, base_blk:base_blk + nblk, :])
                    nc.vector.tensor_tensor(stt[:, :nblk, :], in0, in1,
                                            op=AX.is_equal)
                    chunks[key] = (gt, stt)
                    return chunks[key]

                def chunk_spans(off, nblk):
                    out = []
                    b = off
                    while b < off + nblk:
                        ci = b // CB
                        b1 = min(off + nblk, (ci + 1) * CB)
                        out.append((ci, b - ci * CB, b1 - ci * CB))
                        b = b1
                    return out

                for w in range(W):
                    sl = slice(w * P, (w + 1) * P)
                    pa = pp.tile([P, D], F32, tag="ps_agg", name=f"agg{li}_{w}")
                    spans = [(g, s) for g in range(G)
                             for s in chunk_spans(OFF[g][w], BLK[g][w])]
                    nmm = sum(s[2] - s[1] for _, s in spans)
                    # self-loop folded into psum: pa = hpre[w] + sum S.gt
                    nc.tensor.matmul(pa[:], ident_t[:], hpre[li][:, w, :],
                                     start=True, stop=False)
                    mi = 0
                    for hs, (ci, b0, b1) in spans:
                        gt, stt = _get_chunk(hs, ci)
                        for b in range(b0, b1):
                            nc.tensor.matmul(pa[:], stt[:, b, :], gt[:, b, :],
                                             start=False,
                                             stop=(mi == nmm - 1))
                            mi += 1
                    tf2 = wp.tile([P, D], F32, tag="tf2", name=f"tf2{li}_{w}",
                                   bufs=2)
                    nc.vector.scalar_tensor_tensor(
                        tf2[:], pa[:], dinv_t[:, w:w + 1], br[:],
                        op0=AX.mult, op1=AX.add)
                    hb = wp.tile([P, D], BF16, tag="hb2", name=f"chb{li}_{w}",
                                  bufs=2)
                    nc.scalar.activation(hb[:], tf2[:], AFT.Relu,
                                         scale=valid_t[:, w:w + 1])
                    for r in range(2):
                        pt = pp.tile([P, P], BF16, tag="pt",
                                     name=f"cpt{li}_{w}_{r}")
                        nc.tensor.transpose(pt[:], hb[:, r * P:(r + 1) * P],
                                            ident_t[:])
                        nc.vector.tensor_copy(HT_out[:, r, sl], pt[:])
                    if li == 0:
                        ps2 = pp.tile([P, D], F32, tag="ps_pre",
                                      name=f"pre1_{w}")
                        for r in range(2):
                            nc.tensor.matmul(ps2[:], HT_out[:, r, sl],
                                             w2_t[:, r, :],
                                             start=(r == 0), stop=(r == 1))
                        nc.scalar.activation(hpre[1][:, w, :], ps2[:], AFT.Copy,
                                             scale=dinv_t[:, w:w + 1])
                        if w in send_at:
                            send(1, send_at[w])
                    else:
                        # decoder mlp1 per window (feat-major) + BN stats
                        for fb in range(2):
                            pm = pp.tile([P, P], F32, tag=f"pm{fb}",
                                         name=f"pm{fb}_{w}")
                            for r in range(2):
                                nc.tensor.matmul(
                                    pm[:], wm1_t[:, r, fb * P:(fb + 1) * P],
                                    HT_out[:, r, sl],
                                    start=(r == 0), stop=(r == 1))
                            nc.vector.tensor_scalar(
                                h3[:, fb, sl], pm[:], 1.0, 0.0, op0=AX.mult,
                                op1=AX.add, accum_out=sumps[:, fb, w:w + 1])
                            scr = wp.tile([P, P], F32, tag=f"scr{fb}",
                                          name=f"scr{fb}_{w}", bufs=2)
                            nc.vector.scalar_tensor_tensor(
                                scr[:], h3[:, fb, sl], 1.0, h3[:, fb, sl],
                                op0=AX.mult, op1=AX.mult,
                                accum_out=sqps[:, fb, w:w + 1])

        # ── decoder: BN + relu + mlp2 + softmax (mlp1 ran inside layer 2)
        ht4 = htp.tile([P, 2, NP], BF16, tag="ht", name="ht4")
        with tc.tile_pool(name="dec", bufs=2) as wp, \
             tc.tile_pool(name="decp", bufs=2, space="PSUM") as pp, \
             tc.tile_pool(name="st1", bufs=1) as sp:
            sums = sp.tile([P, 2], F32, tag="sums")
            sqs = sp.tile([P, 2], F32, tag="sqs")
            for fb in range(2):
                nc.vector.reduce_sum(sums[:, fb:fb + 1], sumps[:, fb, :],
                                     axis=mybir.AxisListType.X)
                nc.vector.reduce_sum(sqs[:, fb:fb + 1], sqps[:, fb, :],
                                     axis=mybir.AxisListType.X)
            bnio = sp.tile([P, 4], F32, tag="bnio")
            nc.vector.tensor_copy(bnio[:, 0:2], sums[:])
            nc.vector.tensor_copy(bnio[:, 2:4], sqs[:])
            nc.sync.dma_start(bn_in[:], bnio[:])
            nc.gpsimd.collective_compute(
                "AllReduce", AX.add, ins=[bn_in[:]], outs=[bn_out[:]],
                replica_groups=RG)
            bns = sp.tile([P, 4], F32, tag="bns")
            nc.sync.dma_start(bns[:], bn_out[:])

            mu = sp.tile([P, 2], F32, tag="mu")
            nc.vector.tensor_scalar(mu[:], bns[:, 0:2], ninv, None, op0=AX.mult)
            msq = sp.tile([P, 2], F32, tag="msq")
            nc.vector.tensor_tensor(msq[:], mu[:], mu[:], op=AX.mult)
            var = sp.tile([P, 2], F32, tag="var")
            nc.vector.scalar_tensor_tensor(var[:], bns[:, 2:4], ninv, msq[:],
                                           op0=AX.mult, op1=AX.subtract)
            vae = sp.tile([P, 2], F32, tag="vae")
            nc.vector.tensor_scalar(vae[:], var[:], BN_EPS, None, op0=AX.add)
            sd = sp.tile([P, 2], F32, tag="sd")
            nc.scalar.activation(sd[:], vae[:], AFT.Sqrt)
            rstd = sp.tile([P, 2], F32, tag="rstd")
            nc.vector.reciprocal(rstd[:], sd[:])
            A = sp.tile([P, 2], F32, tag="A")
            nc.vector.tensor_tensor(A[:], rstd[:], gam_t[:], op=AX.mult)
            tb = sp.tile([P, 2], F32, tag="tb")
            nc.vector.tensor_tensor(tb[:], mu[:], A[:], op=AX.mult)
            B = sp.tile([P, 2], F32, tag="B")
            nc.vector.tensor_tensor(B[:], bet_t[:], tb[:], op=AX.subtract)

            lg = sp.tile([P, W, C], F32, tag="lg")
            ex = sp.tile([P, W, C], F32, tag="ex")
            rs = sp.tile([P, W], F32, tag="rs")
            ri = sp.tile([P, W], F32, tag="ri")
            outst = sp.tile([P, W, C], F32, tag="outst")
            TW = 12
            for wc0 in range(0, W, TW):
                wc1 = min(W, wc0 + TW)
                csl = slice(wc0 * P, wc1 * P)
                for fb in range(2):
                    nc.scalar.activation(ht4[:, fb, csl], h3[:, fb, csl],
                                         AFT.Relu, bias=B[:, fb:fb + 1],
                                         scale=A[:, fb:fb + 1])
                for w in range(wc0, wc1):
                    sl = slice(w * P, (w + 1) * P)
                    pl = pp.tile([P, C], F32, tag="ps_lg", name=f"plg{w}")
                    for r in range(2):
                        nc.tensor.matmul(pl[:], ht4[:, r, sl], wm2_t[:, r, :],
                                         start=(r == 0), stop=(r == 1))
                    nc.vector.scalar_tensor_tensor(lg[:, w, :], pl[:], 1.0,
                                                   bcr_t[:],
                                                   op0=AX.mult, op1=AX.add)
                nc.scalar.activation(
                    ex[:, wc0:wc1, :].rearrange("p w c -> p (w c)"),
                    lg[:, wc0:wc1, :].rearrange("p w c -> p (w c)"), AFT.Exp)
                nc.vector.reduce_sum(rs[:, wc0:wc1], ex[:, wc0:wc1, :],
                                     axis=mybir.AxisListType.X)
                nc.vector.reciprocal(ri[:, wc0:wc1], rs[:, wc0:wc1])
                for w in range(wc0, wc1):
                    nc.vector.tensor_scalar(outst[:, w, :], ex[:, w, :],
                                            ri[:, w:w + 1], None, op0=AX.mult)
                nc.sync.dma_start(out_d[:, wc0:wc1, :], outst[:, wc0:wc1, :])

    nc.compile()
    return nc


# ──────────────────────────────────────────────────────────────────────
# NTFF profiling shim (only needed when TRACE)
# ──────────────────────────────────────────────────────────────────────

def _install_hook():
    if "antenv.axon_hooks" in sys.modules:
        return
    so_path = "/opt/axon/libaxon_pjrt.so"
    holder = {"hook": None}
    mod = types.ModuleType("antenv.axon_hooks")
    mod.set_axon_ntff_profile_hook = lambda h: holder.__setitem__("hook", h)
    mod.get_axon_ntff_profile_hook = lambda: holder["hook"]
    sys.modules["antenv.axon_hooks"] = mod
    try:
        import antenv
        antenv.axon_hooks = mod
    except ImportError:
        pass
    try:
        lib = ctypes.CDLL(so_path)
        lib.axon_start_nrt_profile.argtypes = [ctypes.POINTER(ctypes.c_int64),
                                               ctypes.c_size_t]
        lib.axon_start_nrt_profile.restype = ctypes.c_int64
        lib.axon_stop_nrt_profile.argtypes = [ctypes.c_char_p]
        lib.axon_stop_nrt_profile.restype = ctypes.c_int64

        @contextlib.contextmanager
        def _hook(output_dir, device_ids):
            import jax
            jax.devices()
            if device_ids:
                ids = (ctypes.c_int64 * len(device_ids))(*device_ids)
                rc = lib.axon_start_nrt_profile(ids, len(device_ids))
            else:
                rc = lib.axon_start_nrt_profile(None, 0)
            if rc != 0:
                raise RuntimeError(f"axon_start_nrt_profile rc={rc}")
            try:
                yield
            finally:
                nf = lib.axon_stop_nrt_profile(str(output_dir).encode())
                if nf < 0:
                    raise RuntimeError(f"axon_stop_nrt_profile rc={nf}")

        holder["hook"] = _hook
    except OSError:
        pass


# ──────────────────────────────────────────────────────────────────────
# entry point
# ──────────────────────────────────────────────────────────────────────

def kernel(x, edge_index, lin_w, conv1_w, conv1_b, conv2_w, conv2_b,
           mlp1_w, mlp1_b, bn_gamma, bn_beta, mlp2_w, mlp2_b):
    x = np.asarray(x, np.float32)
    n, g = x.shape
    D = int(np.asarray(lin_w).shape[1])
    C = int(np.asarray(mlp2_w).shape[1])
    KENC = _cdiv(g, 256) * 256
    K2E = KENC // 256

    plan = _plan(n, np.asarray(edge_index))
    NP, W, BTOT = plan["NP"], plan["W"], plan["BTOT"]

    key = (n, g, D, C, NP,
           tuple(tuple(b) for b in plan["BLK"]), tuple(plan["GB"]))
    if key not in _CACHE:
        _CACHE[key] = _build(1.0 / float(n), NP, W, plan["GB"], plan["GW"],
                             plan["BLK"], plan["OFF"], plan["NST"],
                             plan["SBASE"], BTOT, plan["MAXB"],
                             K2E, D, C)
    nc = _CACHE[key]

    shared = {
        "wlin": _pack_k(lin_w, KENC),
        "w1": _pack_k(conv1_w, D),
        "w2": _pack_k(conv2_w, D),
        "wm1": _pack_k(mlp1_w, D),
        "wm2": _pack_k(mlp2_w, D),
        "b1r": np.ascontiguousarray(
            np.broadcast_to(np.asarray(conv1_b, np.float32), (P, D))),
        "b2r": np.ascontiguousarray(
            np.broadcast_to(np.asarray(conv2_b, np.float32), (P, D))),
        "bcr": np.ascontiguousarray(
            np.broadcast_to(np.asarray(mlp2_b, np.float32), (P, C))),
        "gam": np.ascontiguousarray(
            np.asarray(bn_gamma, np.float32).reshape(2, P).T),
        "bet": np.ascontiguousarray(
            np.asarray(bn_beta, np.float32).reshape(2, P).T),
        "ident": np.eye(P, dtype=np.float32).astype(ml_dtypes.bfloat16),
        "iota": np.ascontiguousarray(np.broadcast_to(
            np.arange(P, dtype=np.float32), (P, plan["MAXB"], P))).astype(
                ml_dtypes.bfloat16),
    }

    in_maps = []
    for c in range(NCORES):
        xs = x[c * NP:(c + 1) * NP]
        if xs.shape[0] < NP:
            xs = np.vstack([xs, np.zeros((NP - xs.shape[0], g), np.float32)])
        xt = _pack_k(np.ascontiguousarray(xs.T), KENC)
        in_maps.append(dict(shared,
                            xt=xt,
                            gidx=plan["gidx"][c],
                            dl=plan["dl"][c],
                            dinv=plan["dinv"][c],
                            valid=plan["valid"][c]))

    if TRACE:
        _install_hook()
        res = run_bass_kernel_spmd(nc, in_maps, core_ids=list(range(NCORES)),
                                   trace=True, **TRACE_KW)
        LAST["exec_time_ns"] = res.exec_time_ns
        LAST["res"] = res
    else:
        res = run_bass_kernel_spmd(nc, in_maps, core_ids=list(range(NCORES)))

    parts = []
    for c in range(NCORES):
        o = np.asarray(res.results[c]["out"])            # [P, W, C]
        parts.append(np.ascontiguousarray(o.transpose(1, 0, 2)).reshape(NP, C))
    return np.concatenate(parts, axis=0)[:n].astype(np.float32)



# revision 83
# speedup vs baseline: 1.3863x; 1.0102x over previous
"""Trainium2 Bass kernel for CelltypeDeconvolver (GCN message passing).

Runs SPMD on 8 NeuronCores. Nodes are partitioned across cores. Per GCN
layer each core computes h_pre = H @ W for its nodes (scaled by
dinv[src]); the dinv-scaled features are exchanged in three pipelined
AllGather window-groups (each fired as soon as its producer windows
finish, overlapping the next stage), landing in replicated per-group
DRAM tables. Edge source rows are then dma_gathered (software DGE,
8-block chunks rotated over the 4 swdge queues, sized so each chunk's
descriptors fit the 128-slot ring) and segment-reduced on the
TensorEngine with 0/1 fp8 selection matrices generated on-device
(is_equal against an iota tile). Self-loops are folded into the PSUM
accumulation via an identity matmul; the next stage's pre-matmul
(conv2 / decoder mlp1 + BN stats) is interleaved per window so the
BN AllReduce fires immediately when conv2 drains. Epilogue PSUM reads
run on the Activation engine to keep the Vector engine free for mask
generation. Graph structure (edge bucketing, degrees, padding) is
prepared host-side in numpy; all float math happens on-device.
"""

import contextlib
import ctypes
import os
import sys
import types

import numpy as np

for _p in ("/opt/trn_rl_repo",):
    if os.path.isdir(_p) and _p not in sys.path:
        sys.path.append(_p)

import ml_dtypes

import concourse.bass as bass
import concourse.bacc as bacc
import concourse.mybir as mybir
from concourse import library_config
from concourse.tile import TileContext
from concourse.bass_utils import run_bass_kernel_spmd

BF16 = mybir.dt.bfloat16
F32 = mybir.dt.float32
FP8 = mybir.dt.float8e4
I16 = mybir.dt.int16
AX = mybir.AluOpType
AFT = mybir.ActivationFunctionType

NCORES = 8
P = 128
BN_EPS = 1e-5
CB = 8             # gather/S chunk size in 128-slot blocks

TRACE = False
TRACE_KW = {}
LAST = {}
_CACHE = {}


def _pack_k(w, kpad):
    """[K, N] f32 -> [K2, 128, 2, N] bf16 packed (k = k2*256 + r*128 + p)."""
    w = np.asarray(w, np.float32)
    k, n = w.shape
    wp = np.zeros((kpad, n), np.float32)
    wp[:k] = w
    k2 = kpad // 256
    return np.ascontiguousarray(
        wp.reshape(k2, 2, P, n).transpose(0, 2, 1, 3)).astype(ml_dtypes.bfloat16)


def _cdiv(a, b):
    return (a + b - 1) // b


# ──────────────────────────────────────────────────────────────────────
# host-side plan: shard nodes, bucket edges, build index / S arrays
# ──────────────────────────────────────────────────────────────────────

def _plan(n, edge_index):
    NP = _cdiv(_cdiv(n, NCORES), P) * P        # nodes per core (multiple of 128)
    W = NP // P                                 # dst windows per core
    # src window groups: first fires its halo exchange earliest, so keep it
    # small; each group's table must stay int16-indexable (<= 32767 rows).
    g1 = _cdiv(W, 4)
    g3 = _cdiv(W, 4)
    GB = [0, g1, W - g3, W]                     # group bounds
    G = len(GB) - 1
    GW = [GB[i + 1] - GB[i] for i in range(G)]  # group widths
    assert all(NCORES * gw * P <= 32767 for gw in GW)

    src = np.asarray(edge_index[0], np.int64)
    dst = np.asarray(edge_index[1], np.int64)
    deg = np.bincount(dst, minlength=n).astype(np.float32) + 1.0
    dinv = (1.0 / np.sqrt(deg)).astype(np.float32)

    c_arr = dst // NP
    w_arr = (dst % NP) // P
    dl_arr = (dst % P).astype(np.int64)
    # src node -> (window group, row in that group's table)
    # group-g table layout = [(c p w), D] over that group's windows
    cs = src // NP
    ii = src % NP
    ws = ii // P
    psrc = ii % P
    grp = np.searchsorted(np.asarray(GB[1:]), ws, side="right").astype(np.int64)
    gw_arr = np.asarray(GW, np.int64)[grp]
    gb_arr = np.asarray(GB[:-1], np.int64)[grp]
    row = (cs * (P * gw_arr) + psrc * gw_arr + (ws - gb_arr)).astype(np.int64)

    order = np.lexsort((row, w_arr, c_arr, grp))
    c_s, w_s, h_s = c_arr[order], w_arr[order], grp[order]
    row_s, dl_s = row[order], dl_arr[order]

    key = ((h_s * NCORES + c_s) * W + w_s)
    cnt = np.bincount(key, minlength=G * NCORES * W).reshape(G, NCORES, W)
    starts = np.zeros(G * NCORES * W + 1, np.int64)
    np.cumsum(cnt.reshape(-1), out=starts[1:])

    # per-group per-window block counts and stream offsets
    BLK = [np.maximum(_cdiv(cnt[g].max(axis=0), P), 1) for g in range(G)]
    OFF = []
    for g in range(G):
        o = np.zeros(W + 1, np.int64)
        np.cumsum(BLK[g], out=o[1:])
        OFF.append(o)
    NST = [int(OFF[g][-1]) for g in range(G)]   # blocks per stream
    SBASE = [int(sum(NST[:g])) for g in range(G)]
    BTOT = int(sum(NST))
    MAXB = max(int(max(b.max() for b in BLK)), CB)

    gidx_list, dl_list = [], []
    for c in range(NCORES):
        fidx = np.zeros(BTOT * P, np.int16)
        fdl = np.full(BTOT * P, -1, np.int64)
        for h in range(G):
            for w in range(W):
                k = (h * NCORES + c) * W + w
                s0, s1 = starts[k], starts[k + 1]
                m = s1 - s0
                if m == 0:
                    continue
                base = (SBASE[h] + OFF[h][w]) * P
                fidx[base:base + m] = row_s[s0:s1].astype(np.int16)
                fdl[base:base + m] = dl_s[s0:s1]
        gidx_list.append(np.ascontiguousarray(
            np.tile(fidx.reshape(-1, 16).T, (NCORES, 1))))
        dl = np.full((P, BTOT, 1), -1.0, np.float32)
        pos = np.nonzero(fdl >= 0)[0]
        dl[pos % P, pos // P, 0] = fdl[pos]
        dl_list.append(dl.astype(ml_dtypes.bfloat16))

    dinv_t, valid_t = [], []
    for c in range(NCORES):
        g = c * NP + (np.arange(P)[:, None] + P * np.arange(W)[None, :])
        real = g < n
        dv = np.zeros((P, W), np.float32)
        dv[real] = dinv[g[real]]
        dinv_t.append(dv)
        valid_t.append(real.astype(np.float32))

    return dict(n=n, NP=NP, W=W, GB=GB, GW=GW,
                BLK=[[int(v) for v in b] for b in BLK],
                OFF=[[int(v) for v in o] for o in OFF],
                NST=NST, SBASE=SBASE, BTOT=BTOT, MAXB=MAXB,
                gidx=gidx_list, dl=dl_list,
                dinv=dinv_t, valid=valid_t)


# ──────────────────────────────────────────────────────────────────────
# device program
# ──────────────────────────────────────────────────────────────────────

def _build(ninv, NP, W, GB, GW, BLK, OFF, NST, SBASE, BTOT,
           MAXB, K2E, D, C):
    RG = [list(range(NCORES))]
    G = len(GW)
    nc = bacc.Bacc("TRN2", num_devices=NCORES, num_swdge_queues=4)

    xt_d = nc.dram_tensor("xt", [K2E, P, 2, NP], BF16, kind="ExternalInput")
    wlin_d = nc.dram_tensor("wlin", [K2E, P, 2, D], BF16, kind="ExternalInput")
    w1_d = nc.dram_tensor("w1", [1, P, 2, D], BF16, kind="ExternalInput")
    w2_d = nc.dram_tensor("w2", [1, P, 2, D], BF16, kind="ExternalInput")
    wm1_d = nc.dram_tensor("wm1", [1, P, 2, D], BF16, kind="ExternalInput")
    wm2_d = nc.dram_tensor("wm2", [1, P, 2, C], BF16, kind="ExternalInput")
    b1r_d = nc.dram_tensor("b1r", [P, D], F32, kind="ExternalInput")
    b2r_d = nc.dram_tensor("b2r", [P, D], F32, kind="ExternalInput")
    bcr_d = nc.dram_tensor("bcr", [P, C], F32, kind="ExternalInput")
    gam_d = nc.dram_tensor("gam", [P, 2], F32, kind="ExternalInput")
    bet_d = nc.dram_tensor("bet", [P, 2], F32, kind="ExternalInput")
    ident_d = nc.dram_tensor("ident", [P, P], BF16, kind="ExternalInput")
    gidx_d = nc.dram_tensor("gidx", [P, BTOT * 8], I16, kind="ExternalInput")
    dl_d = nc.dram_tensor("dl", [P, BTOT, 1], BF16, kind="ExternalInput")
    iota_d = nc.dram_tensor("iota", [P, MAXB, P], BF16, kind="ExternalInput")
    dinv_d = nc.dram_tensor("dinv", [P, W], F32, kind="ExternalInput")
    valid_d = nc.dram_tensor("valid", [P, W], F32, kind="ExternalInput")
    out_d = nc.dram_tensor("out", [P, W, C], F32, kind="ExternalOutput")

    ag_ins = [[nc.dram_tensor(f"ag_in{g}_{i}", [P, GW[g] * D], BF16)
               for g in range(G)] for i in range(2)]
    tables = [[nc.dram_tensor(f"table{g}_{i}", [NCORES * GW[g] * P, D], BF16,
                              addr_space="Shared") for g in range(G)]
              for i in range(2)]
    bn_in = nc.dram_tensor("bn_in", [P, 4], F32)
    bn_out = nc.dram_tensor("bn_out", [P, 4], F32, addr_space="Shared")

    CH = 7                                     # encoder windows per x-chunk

    with TileContext(nc) as tc, contextlib.ExitStack() as ctx:
        cp = ctx.enter_context(tc.tile_pool(name="const", bufs=1))
        big = ctx.enter_context(tc.tile_pool(name="big", bufs=2))
        htp = ctx.enter_context(tc.tile_pool(name="htp", bufs=2))

        nc.gpsimd.load_library(library_config.mlp)

        def cload(dram, shape, dtype, tag, src=None):
            t = cp.tile(shape, dtype, tag=tag, name=tag)
            nc.sync.dma_start(t[:], dram[:] if src is None else src)
            return t

        ident_t = cload(ident_d, [P, P], BF16, "ident")
        w1_t = cload(w1_d, [P, 2, D], BF16, "w1", src=w1_d[0])
        w2_t = cload(w2_d, [P, 2, D], BF16, "w2", src=w2_d[0])
        wm1_t = cload(wm1_d, [P, 2, D], BF16, "wm1", src=wm1_d[0])
        wm2_t = cload(wm2_d, [P, 2, C], BF16, "wm2", src=wm2_d[0])
        b1r_t = cload(b1r_d, [P, D], F32, "b1r")
        b2r_t = cload(b2r_d, [P, D], F32, "b2r")
        bcr_t = cload(bcr_d, [P, C], F32, "bcr")
        gam_t = cload(gam_d, [P, 2], F32, "gam")
        bet_t = cload(bet_d, [P, 2], F32, "bet")
        gidx_t = cload(gidx_d, [P, BTOT * 8], I16, "gidx")
        dl_t = cload(dl_d, [P, BTOT, 1], BF16, "dl")
        iota_t = cload(iota_d, [P, MAXB, P], BF16, "iota")
        dinv_t = cload(dinv_d, [P, W], F32, "dinv")
        valid_t = cload(valid_d, [P, W], F32, "valid")

        # persistent activations: ht slots rotate h0T -> h1T -> h2T -> h4T
        ht = [htp.tile([P, 2, NP], BF16, tag="ht", name=f"ht{i}")
              for i in range(3)]
        # hpre0 / hpre1 / h3 share one 2-deep rotation: h3 (layer-2 output)
        # reuses hpre0's buffer, whose last reader is layer 1's self-loop.
        hpre = [big.tile([P, W, D], BF16, tag="bigbuf", name=f"hpre{i}")
                for i in range(2)]
        h3 = big.tile([P, 2, NP], BF16, tag="bigbuf", name="h3")
        sumps = cp.tile([P, 2, W], F32, tag="sumps")
        sqps = cp.tile([P, 2, W], F32, tag="sqps")

        def send(li, g):
            nc.sync.dma_start(ag_ins[li][g][:],
                              hpre[li][:, GB[g]:GB[g + 1], :])
            nc.gpsimd.collective_compute(
                "AllGather", AX.bypass, ins=[ag_ins[li][g][:]],
                outs=[tables[li][g][:]], replica_groups=RG)

        send_at = {GB[g + 1] - 1: g for g in range(G)}

        # ── encoder: h0 = x @ lin_w (node-major) → transpose → ht[0],
        #    with conv1's pre-matmul interleaved per window
        with tc.tile_pool(name="encw", bufs=2) as wp, \
             tc.tile_pool(name="encp", bufs=2, space="PSUM") as pp, \
             tc.tile_pool(name="xtp", bufs=2) as xtp:
            wlin_t = []
            for k2 in range(K2E):
                t = cp.tile([P, 2, D], BF16, tag=f"wlin{k2}", name=f"wlin{k2}")
                nc.sync.dma_start(t[:], wlin_d[k2])
                wlin_t.append(t)
            for wc in range(_cdiv(W, CH)):
                ws, we = wc * CH, min(W, (wc + 1) * CH)
                xtc = []
                for k2 in range(K2E):
                    t = xtp.tile([P, 2, CH * P], BF16, tag=f"xtc{k2}",
                                 name=f"xtc{k2}_{wc}")
                    eng = nc.sync if k2 % 2 == 0 else nc.scalar
                    eng.dma_start(t[:, :, :(we - ws) * P],
                                  xt_d[k2][:, :, ws * P:we * P])
                    xtc.append(t)
                for w in range(ws, we):
                    lsl = slice((w - ws) * P, (w - ws + 1) * P)
                    sl = slice(w * P, (w + 1) * P)
                    ps = pp.tile([P, D], F32, tag="ps", name=f"eps{w}")
                    for k2 in range(K2E):
                        for r in range(2):
                            nc.tensor.matmul(
                                ps[:], xtc[k2][:, r, lsl], wlin_t[k2][:, r, :],
                                start=(k2 == 0 and r == 0),
                                stop=(k2 == K2E - 1 and r == 1))
                    hb = wp.tile([P, D], BF16, tag="hb", name=f"ehb{w}")
                    nc.vector.tensor_copy(hb[:], ps[:])
                    for r in range(2):
                        pt = pp.tile([P, P], BF16, tag="pt", name=f"ept{w}_{r}")
                        nc.tensor.transpose(pt[:], hb[:, r * P:(r + 1) * P],
                                            ident_t[:])
                        nc.vector.tensor_copy(ht[0][:, r, sl], pt[:])
                    ps2 = pp.tile([P, D], F32, tag="ps_pre", name=f"pre0_{w}")
                    for r in range(2):
                        nc.tensor.matmul(ps2[:], ht[0][:, r, sl], w1_t[:, r, :],
                                         start=(r == 0), stop=(r == 1))
                    nc.scalar.activation(hpre[0][:, w, :], ps2[:], AFT.Copy,
                                         scale=dinv_t[:, w:w + 1])
                    if w in send_at:
                        send(0, send_at[w])

        # ── conv layers (layer li consumes tableA/B[li]; the next stage's
        #    pre-matmul + halo send are interleaved into this layer's loop)
        for li in range(2):
            HT_out = ht[li + 1]
            br = b1r_t if li == 0 else b2r_t
            with tc.tile_pool(name=f"cw{li}", bufs=3) as wp, \
                 tc.tile_pool(name=f"cp{li}", bufs=2, space="PSUM") as pp:

                chunks = {}
                qc = [0]

                def _get_chunk(hs, ci, chunks=chunks, wp=wp, li=li):
                    key = (hs, ci)
                    if key in chunks:
                        return chunks[key]
                    nstream = NST[hs]
                    base_blk = SBASE[hs] + ci * CB
                    nblk = min(CB, nstream - ci * CB)
                    nn = nblk * P
                    gt = wp.tile([P, CB, D], BF16, tag=f"gt{hs}",
                                 name=f"gt{li}_{hs}_{ci}", bufs=5)
                    stt = wp.tile([P, CB, P], FP8, tag=f"st{hs}",
                                  name=f"st{li}_{hs}_{ci}", bufs=5)
                    tb = tables[li][hs][:]
                    nc.gpsimd.dma_gather(
                        gt[:, :nblk, :], tb,
                        gidx_t[:, base_blk * 8:(base_blk + nblk) * 8],
                        nn, nn, D, single_packet=True,
                        queue_num=qc[0] % 4)
                    qc[0] += 1
                    # build the 0/1 selection block on-device: S[e,b,j] =
                    # (j == dst_lane[e,b]); padding slots have dl = -1.
                    in0, in1 = bass.broadcast_tensor_aps(
                        iota_t[:, :nblk, :],
                        dl_t[:, base_blk:base_blk + nblk, :])
                    nc.vector.tensor_tensor(stt[:, :nblk, :], in0, in1,
                                            op=AX.is_equal)
                    chunks[key] = (gt, stt)
                    return chunks[key]

                def chunk_spans(off, nblk):
                    out = []
                    b = off
                    while b < off + nblk:
                        ci = b // CB
                        b1 = min(off + nblk, (ci + 1) * CB)
                        out.append((ci, b - ci * CB, b1 - ci * CB))
                        b = b1
                    return out

                # issue a few chunks of the early-landing streams before
                # the last group's first gather (which waits on its
                # AllGather) head-of-line blocks the in-order gpsimd
                for g in range(G - 1):
                    for ci in range(min(2 + g, _cdiv(NST[g], CB))):
                        _get_chunk(g, ci)

                for w in range(W):
                    sl = slice(w * P, (w + 1) * P)
                    pa = pp.tile([P, D], F32, tag="ps_agg", name=f"agg{li}_{w}")
                    spans = [(g, s) for g in range(G)
                             for s in chunk_spans(OFF[g][w], BLK[g][w])]
                    nmm = sum(s[2] - s[1] for _, s in spans)
                    # self-loop folded into psum: pa = hpre[w] + sum S.gt
                    nc.tensor.matmul(pa[:], ident_t[:], hpre[li][:, w, :],
                                     start=True, stop=False)
                    mi = 0
                    for hs, (ci, b0, b1) in spans:
                        gt, stt = _get_chunk(hs, ci)
                        for b in range(b0, b1):
                            nc.tensor.matmul(pa[:], stt[:, b, :], gt[:, b, :],
                                             start=False,
                                             stop=(mi == nmm - 1))
                            mi += 1
                    tf2 = wp.tile([P, D], F32, tag="tf2", name=f"tf2{li}_{w}",
                                   bufs=2)
                    nc.vector.scalar_tensor_tensor(
                        tf2[:], pa[:], dinv_t[:, w:w + 1], br[:],
                        op0=AX.mult, op1=AX.add)
                    hb = wp.tile([P, D], BF16, tag="hb2", name=f"chb{li}_{w}",
                                  bufs=2)
                    nc.scalar.activation(hb[:], tf2[:], AFT.Relu,
                                         scale=valid_t[:, w:w + 1])
                    for r in range(2):
                        pt = pp.tile([P, P], BF16, tag="pt",
                                     name=f"cpt{li}_{w}_{r}")
                        nc.tensor.transpose(pt[:], hb[:, r * P:(r + 1) * P],
                                            ident_t[:])
                        nc.vector.tensor_copy(HT_out[:, r, sl], pt[:])
                    if li == 0:
                        ps2 = pp.tile([P, D], F32, tag="ps_pre",
                                      name=f"pre1_{w}")
                        for r in range(2):
                            nc.tensor.matmul(ps2[:], HT_out[:, r, sl],
                                             w2_t[:, r, :],
                                             start=(r == 0), stop=(r == 1))
                        nc.scalar.activation(hpre[1][:, w, :], ps2[:], AFT.Copy,
                                             scale=dinv_t[:, w:w + 1])
                        if w in send_at:
                            send(1, send_at[w])
                    else:
                        # decoder mlp1 per window (feat-major) + BN stats
                        for fb in range(2):
                            pm = pp.tile([P, P], F32, tag=f"pm{fb}",
                                         name=f"pm{fb}_{w}")
                            for r in range(2):
                                nc.tensor.matmul(
                                    pm[:], wm1_t[:, r, fb * P:(fb + 1) * P],
                                    HT_out[:, r, sl],
                                    start=(r == 0), stop=(r == 1))
                            nc.vector.tensor_scalar(
                                h3[:, fb, sl], pm[:], 1.0, 0.0, op0=AX.mult,
                                op1=AX.add, accum_out=sumps[:, fb, w:w + 1])
                            scr = wp.tile([P, P], F32, tag=f"scr{fb}",
                                          name=f"scr{fb}_{w}", bufs=2)
                            nc.vector.scalar_tensor_tensor(
                                scr[:], h3[:, fb, sl], 1.0, h3[:, fb, sl],
                                op0=AX.mult, op1=AX.mult,
                                accum_out=sqps[:, fb, w:w + 1])

        # ── decoder: BN + relu + mlp2 + softmax (mlp1 ran inside layer 2)
        ht4 = htp.tile([P, 2, NP], BF16, tag="ht", name="ht4")
        with tc.tile_pool(name="dec", bufs=2) as wp, \
             tc.tile_pool(name="decp", bufs=2, space="PSUM") as pp, \
             tc.tile_pool(name="st1", bufs=1) as sp:
            sums = sp.tile([P, 2], F32, tag="sums")
            sqs = sp.tile([P, 2], F32, tag="sqs")
            for fb in range(2):
                nc.vector.reduce_sum(sums[:, fb:fb + 1], sumps[:, fb, :],
                                     axis=mybir.AxisListType.X)
                nc.vector.reduce_sum(sqs[:, fb:fb + 1], sqps[:, fb, :],
                                     axis=mybir.AxisListType.X)
            bnio = sp.tile([P, 4], F32, tag="bnio")
            nc.vector.tensor_copy(bnio[:, 0:2], sums[:])
            nc.vector.tensor_copy(bnio[:, 2:4], sqs[:])
            nc.sync.dma_start(bn_in[:], bnio[:])
            nc.gpsimd.collective_compute(
                "AllReduce", AX.add, ins=[bn_in[:]], outs=[bn_out[:]],
                replica_groups=RG)
            bns = sp.tile([P, 4], F32, tag="bns")
            nc.sync.dma_start(bns[:], bn_out[:])

            mu = sp.tile([P, 2], F32, tag="mu")
            nc.vector.tensor_scalar(mu[:], bns[:, 0:2], ninv, None, op0=AX.mult)
            msq = sp.tile([P, 2], F32, tag="msq")
            nc.vector.tensor_tensor(msq[:], mu[:], mu[:], op=AX.mult)
            var = sp.tile([P, 2], F32, tag="var")
            nc.vector.scalar_tensor_tensor(var[:], bns[:, 2:4], ninv, msq[:],
                                           op0=AX.mult, op1=AX.subtract)
            vae = sp.tile([P, 2], F32, tag="vae")
            nc.vector.tensor_scalar(vae[:], var[:], BN_EPS, None, op0=AX.add)
            sd = sp.tile([P, 2], F32, tag="sd")
            nc.scalar.activation(sd[:], vae[:], AFT.Sqrt)
            rstd = sp.tile([P, 2], F32, tag="rstd")
            nc.vector.reciprocal(rstd[:], sd[:])
            A = sp.tile([P, 2], F32, tag="A")
            nc.vector.tensor_tensor(A[:], rstd[:], gam_t[:], op=AX.mult)
            tb = sp.tile([P, 2], F32, tag="tb")
            nc.vector.tensor_tensor(tb[:], mu[:], A[:], op=AX.mult)
            B = sp.tile([P, 2], F32, tag="B")
            nc.vector.tensor_tensor(B[:], bet_t[:], tb[:], op=AX.subtract)

            lg = sp.tile([P, W, C], F32, tag="lg")
            ex = sp.tile([P, W, C], F32, tag="ex")
            rs = sp.tile([P, W], F32, tag="rs")
            ri = sp.tile([P, W], F32, tag="ri")
            outst = sp.tile([P, W, C], F32, tag="outst")
            TW = 12
            for wc0 in range(0, W, TW):
                wc1 = min(W, wc0 + TW)
                csl = slice(wc0 * P, wc1 * P)
                for fb in range(2):
                    nc.scalar.activation(ht4[:, fb, csl], h3[:, fb, csl],
                                         AFT.Relu, bias=B[:, fb:fb + 1],
                                         scale=A[:, fb:fb + 1])
                for w in range(wc0, wc1):
                    sl = slice(w * P, (w + 1) * P)
                    pl = pp.tile([P, C], F32, tag="ps_lg", name=f"plg{w}")
                    for r in range(2):
                        nc.tensor.matmul(pl[:], ht4[:, r, sl], wm2_t[:, r, :],
                                         start=(r == 0), stop=(r == 1))
                    nc.vector.scalar_tensor_tensor(lg[:, w, :], pl[:], 1.0,
                                                   bcr_t[:],
                                                   op0=AX.mult, op1=AX.add)
                nc.scalar.activation(
                    ex[:, wc0:wc1, :].rearrange("p w c -> p (w c)"),
                    lg[:, wc0:wc1, :].rearrange("p w c -> p (w c)"), AFT.Exp)
                nc.vector.reduce_sum(rs[:, wc0:wc1], ex[:, wc0:wc1, :],
                                     axis=mybir.AxisListType.X)
                nc.vector.reciprocal(ri[:, wc0:wc1], rs[:, wc0:wc1])
                for w in range(wc0, wc1):
                    nc.vector.tensor_scalar(outst[:, w, :], ex[:, w, :],
                                            ri[:, w:w + 1], None, op0=AX.mult)
                nc.sync.dma_start(out_d[:, wc0:wc1, :], outst[:, wc0:wc1, :])

    nc.compile()
    return nc


# ──────────────────────────────────────────────────────────────────────
# NTFF profiling shim (only needed when TRACE)
# ──────────────────────────────────────────────────────────────────────

def _install_hook():
    if "antenv.axon_hooks" in sys.modules:
        return
    so_path = "/opt/axon/libaxon_pjrt.so"
    holder = {"hook": None}
    mod = types.ModuleType("antenv.axon_hooks")
    mod.set_axon_ntff_profile_hook = lambda h: holder.__setitem__("hook", h)
    mod.get_axon_ntff_profile_hook = lambda: holder["hook"]
    sys.modules["antenv.axon_hooks"] = mod
    try:
        import antenv
        antenv.axon_hooks = mod
    except ImportError:
        pass
    try:
        lib = ctypes.CDLL(so_path)
        lib.axon_start_nrt_profile.argtypes = [ctypes.POINTER(ctypes.c_int64),
                                               ctypes.c_size_t]
        lib.axon_start_nrt_profile.restype = ctypes.c_int64
        lib.axon_stop_nrt_profile.argtypes = [ctypes.c_char_p]
        lib.axon_stop_nrt_profile.restype = ctypes.c_int64

        @contextlib.contextmanager
        def _hook(output_dir, device_ids):
            import jax
            jax.devices()
            if device_ids:
                ids = (ctypes.c_int64 * len(device_ids))(*device_ids)
                rc = lib.axon_start_nrt_profile(ids, len(device_ids))
            else:
                rc = lib.axon_start_nrt_profile(None, 0)
            if rc != 0:
                raise RuntimeError(f"axon_start_nrt_profile rc={rc}")
            try:
                yield
            finally:
                nf = lib.axon_stop_nrt_profile(str(output_dir).encode())
                if nf < 0:
                    raise RuntimeError(f"axon_stop_nrt_profile rc={nf}")

        holder["hook"] = _hook
    except OSError:
        pass


# ──────────────────────────────────────────────────────────────────────
# entry point
# ──────────────────────────────────────────────────────────────────────

def kernel(x, edge_index, lin_w, conv1_w, conv1_b, conv2_w, conv2_b,
           mlp1_w, mlp1_b, bn_gamma, bn_beta, mlp2_w, mlp2_b):
    x = np.asarray(x, np.float32)
    n, g = x.shape
    D = int(np.asarray(lin_w).shape[1])
    C = int(np.asarray(mlp2_w).shape[1])
    KENC = _cdiv(g, 256) * 256
    K2E = KENC // 256

    plan = _plan(n, np.asarray(edge_index))
    NP, W, BTOT = plan["NP"], plan["W"], plan["BTOT"]

    key = (n, g, D, C, NP,
           tuple(tuple(b) for b in plan["BLK"]), tuple(plan["GB"]))
    if key not in _CACHE:
        _CACHE[key] = _build(1.0 / float(n), NP, W, plan["GB"], plan["GW"],
                             plan["BLK"], plan["OFF"], plan["NST"],
                             plan["SBASE"], BTOT, plan["MAXB"],
                             K2E, D, C)
    nc = _CACHE[key]

    shared = {
        "wlin": _pack_k(lin_w, KENC),
        "w1": _pack_k(conv1_w, D),
        "w2": _pack_k(conv2_w, D),
        "wm1": _pack_k(mlp1_w, D),
        "wm2": _pack_k(mlp2_w, D),
        "b1r": np.ascontiguousarray(
            np.broadcast_to(np.asarray(conv1_b, np.float32), (P, D))),
        "b2r": np.ascontiguousarray(
            np.broadcast_to(np.asarray(conv2_b, np.float32), (P, D))),
        "bcr": np.ascontiguousarray(
            np.broadcast_to(np.asarray(mlp2_b, np.float32), (P, C))),
        "gam": np.ascontiguousarray(
            np.asarray(bn_gamma, np.float32).reshape(2, P).T),
        "bet": np.ascontiguousarray(
            np.asarray(bn_beta, np.float32).reshape(2, P).T),
        "ident": np.eye(P, dtype=np.float32).astype(ml_dtypes.bfloat16),
        "iota": np.ascontiguousarray(np.broadcast_to(
            np.arange(P, dtype=np.float32), (P, plan["MAXB"], P))).astype(
                ml_dtypes.bfloat16),
    }

    in_maps = []
    for c in range(NCORES):
        xs = x[c * NP:(c + 1) * NP]
        if xs.shape[0] < NP:
            xs = np.vstack([xs, np.zeros((NP - xs.shape[0], g), np.float32)])
        xt = _pack_k(np.ascontiguousarray(xs.T), KENC)
        in_maps.append(dict(shared,
                            xt=xt,
                            gidx=plan["gidx"][c],
                            dl=plan["dl"][c],
                            dinv=plan["dinv"][c],
                            valid=plan["valid"][c]))

    if TRACE:
        _install_hook()
        res = run_bass_kernel_spmd(nc, in_maps, core_ids=list(range(NCORES)),
                                   trace=True, **TRACE_KW)
        LAST["exec_time_ns"] = res.exec_time_ns
        LAST["res"] = res
    else:
        res = run_bass_kernel_spmd(nc, in_maps, core_ids=list(range(NCORES)))

    parts = []
    for c in range(NCORES):
        o = np.asarray(res.results[c]["out"])            # [P, W, C]
        parts.append(np.ascontiguousarray(o.transpose(1, 0, 2)).reshape(NP, C))
    return np.concatenate(parts, axis=0)[:n].astype(np.float32)



# revision 84
# speedup vs baseline: 1.3927x; 1.0046x over previous
"""Trainium2 Bass kernel for CelltypeDeconvolver (GCN message passing).

Runs SPMD on 8 NeuronCores. Nodes are partitioned across cores. Per GCN
layer each core computes h_pre = H @ W for its nodes (scaled by
dinv[src]); the dinv-scaled features are exchanged in three pipelined
AllGather window-groups (each fired as soon as its producer windows
finish, overlapping the next stage), landing in replicated per-group
DRAM tables. Edge source rows are then dma_gathered (software DGE,
8-block chunks rotated over the 4 swdge queues, sized so each chunk's
descriptors fit the 128-slot ring) and segment-reduced on the
TensorEngine with 0/1 fp8 selection matrices generated on-device
(is_equal against an iota tile). Self-loops are folded into the PSUM
accumulation via an identity matmul; the next stage's pre-matmul
(conv2 / decoder mlp1 + BN stats) is interleaved per window so the
BN AllReduce fires immediately when conv2 drains. Epilogue PSUM reads
run on the Activation engine to keep the Vector engine free for mask
generation. Graph structure (edge bucketing, degrees, padding) is
prepared host-side in numpy; all float math happens on-device.
"""

import contextlib
import ctypes
import os
import sys
import types

import numpy as np

for _p in ("/opt/trn_rl_repo",):
    if os.path.isdir(_p) and _p not in sys.path:
        sys.path.append(_p)

import ml_dtypes

import concourse.bass as bass
import concourse.bacc as bacc
import concourse.mybir as mybir
from concourse import library_config
from concourse.tile import TileContext
from concourse.bass_utils import run_bass_kernel_spmd

BF16 = mybir.dt.bfloat16
F32 = mybir.dt.float32
FP8 = mybir.dt.float8e4
I16 = mybir.dt.int16
AX = mybir.AluOpType
AFT = mybir.ActivationFunctionType

NCORES = 8
P = 128
BN_EPS = 1e-5
CB = 8             # gather/S chunk size in 128-slot blocks

TRACE = False
TRACE_KW = {}
LAST = {}
_CACHE = {}


def _pack_k(w, kpad):
    """[K, N] f32 -> [K2, 128, 2, N] bf16 packed (k = k2*256 + r*128 + p)."""
    w = np.asarray(w, np.float32)
    k, n = w.shape
    wp = np.zeros((kpad, n), np.float32)
    wp[:k] = w
    k2 = kpad // 256
    return np.ascontiguousarray(
        wp.reshape(k2, 2, P, n).transpose(0, 2, 1, 3)).astype(ml_dtypes.bfloat16)


def _cdiv(a, b):
    return (a + b - 1) // b


# ──────────────────────────────────────────────────────────────────────
# host-side plan: shard nodes, bucket edges, build index / S arrays
# ──────────────────────────────────────────────────────────────────────

def _plan(n, edge_index):
    NP = _cdiv(_cdiv(n, NCORES), P) * P        # nodes per core (multiple of 128)
    W = NP // P                                 # dst windows per core
    # src window groups: first fires its halo exchange earliest, so keep it
    # small; each group's table must stay int16-indexable (<= 32767 rows).
    g1 = _cdiv(W, 4)
    g3 = _cdiv(W, 4)
    GB = [0, g1, W - g3, W]                     # group bounds
    G = len(GB) - 1
    GW = [GB[i + 1] - GB[i] for i in range(G)]  # group widths
    assert all(NCORES * gw * P <= 32767 for gw in GW)

    src = np.asarray(edge_index[0], np.int64)
    dst = np.asarray(edge_index[1], np.int64)
    deg = np.bincount(dst, minlength=n).astype(np.float32) + 1.0
    dinv = (1.0 / np.sqrt(deg)).astype(np.float32)

    c_arr = dst // NP
    w_arr = (dst % NP) // P
    dl_arr = (dst % P).astype(np.int64)
    # src node -> (window group, row in that group's table)
    # group-g table layout = [(c p w), D] over that group's windows
    cs = src // NP
    ii = src % NP
    ws = ii // P
    psrc = ii % P
    grp = np.searchsorted(np.asarray(GB[1:]), ws, side="right").astype(np.int64)
    gw_arr = np.asarray(GW, np.int64)[grp]
    gb_arr = np.asarray(GB[:-1], np.int64)[grp]
    row = (cs * (P * gw_arr) + psrc * gw_arr + (ws - gb_arr)).astype(np.int64)

    order = np.lexsort((row, w_arr, c_arr, grp))
    c_s, w_s, h_s = c_arr[order], w_arr[order], grp[order]
    row_s, dl_s = row[order], dl_arr[order]

    key = ((h_s * NCORES + c_s) * W + w_s)
    cnt = np.bincount(key, minlength=G * NCORES * W).reshape(G, NCORES, W)
    starts = np.zeros(G * NCORES * W + 1, np.int64)
    np.cumsum(cnt.reshape(-1), out=starts[1:])

    # per-group per-window block counts and stream offsets
    BLK = [np.maximum(_cdiv(cnt[g].max(axis=0), P), 1) for g in range(G)]
    OFF = []
    for g in range(G):
        o = np.zeros(W + 1, np.int64)
        np.cumsum(BLK[g], out=o[1:])
        OFF.append(o)
    NST = [int(OFF[g][-1]) for g in range(G)]   # blocks per stream
    SBASE = [int(sum(NST[:g])) for g in range(G)]
    BTOT = int(sum(NST))
    MAXB = max(int(max(b.max() for b in BLK)), CB)

    gidx_list, dl_list = [], []
    for c in range(NCORES):
        fidx = np.zeros(BTOT * P, np.int16)
        fdl = np.full(BTOT * P, -1, np.int64)
        for h in range(G):
            for w in range(W):
                k = (h * NCORES + c) * W + w
                s0, s1 = starts[k], starts[k + 1]
                m = s1 - s0
                if m == 0:
                    continue
                base = (SBASE[h] + OFF[h][w]) * P
                fidx[base:base + m] = row_s[s0:s1].astype(np.int16)
                fdl[base:base + m] = dl_s[s0:s1]
        gidx_list.append(np.ascontiguousarray(
            np.tile(fidx.reshape(-1, 16).T, (NCORES, 1))))
        dl = np.full((P, BTOT, 1), -1.0, np.float32)
        pos = np.nonzero(fdl >= 0)[0]
        dl[pos % P, pos // P, 0] = fdl[pos]
        dl_list.append(dl.astype(ml_dtypes.bfloat16))

    dinv_t, valid_t = [], []
    for c in range(NCORES):
        g = c * NP + (np.arange(P)[:, None] + P * np.arange(W)[None, :])
        real = g < n
        dv = np.zeros((P, W), np.float32)
        dv[real] = dinv[g[real]]
        dinv_t.append(dv)
        valid_t.append(real.astype(np.float32))

    return dict(n=n, NP=NP, W=W, GB=GB, GW=GW,
                BLK=[[int(v) for v in b] for b in BLK],
                OFF=[[int(v) for v in o] for o in OFF],
                NST=NST, SBASE=SBASE, BTOT=BTOT, MAXB=MAXB,
                gidx=gidx_list, dl=dl_list,
                dinv=dinv_t, valid=valid_t)


# ──────────────────────────────────────────────────────────────────────
# device program
# ──────────────────────────────────────────────────────────────────────

def _build(ninv, NP, W, GB, GW, BLK, OFF, NST, SBASE, BTOT,
           MAXB, K2E, D, C):
    RG = [list(range(NCORES))]
    G = len(GW)
    nc = bacc.Bacc("TRN2", num_devices=NCORES, num_swdge_queues=4)

    xt_d = nc.dram_tensor("xt", [K2E, P, 2, NP], BF16, kind="ExternalInput")
    wlin_d = nc.dram_tensor("wlin", [K2E, P, 2, D], BF16, kind="ExternalInput")
    w1_d = nc.dram_tensor("w1", [1, P, 2, D], BF16, kind="ExternalInput")
    w2_d = nc.dram_tensor("w2", [1, P, 2, D], BF16, kind="ExternalInput")
    wm1_d = nc.dram_tensor("wm1", [1, P, 2, D], BF16, kind="ExternalInput")
    wm2_d = nc.dram_tensor("wm2", [1, P, 2, C], BF16, kind="ExternalInput")
    b1r_d = nc.dram_tensor("b1r", [P, D], F32, kind="ExternalInput")
    b2r_d = nc.dram_tensor("b2r", [P, D], F32, kind="ExternalInput")
    bcr_d = nc.dram_tensor("bcr", [P, C], F32, kind="ExternalInput")
    gam_d = nc.dram_tensor("gam", [P, 2], F32, kind="ExternalInput")
    bet_d = nc.dram_tensor("bet", [P, 2], F32, kind="ExternalInput")
    ident_d = nc.dram_tensor("ident", [P, P], BF16, kind="ExternalInput")
    gidx_d = nc.dram_tensor("gidx", [P, BTOT * 8], I16, kind="ExternalInput")
    dl_d = nc.dram_tensor("dl", [P, BTOT, 1], BF16, kind="ExternalInput")
    iota_d = nc.dram_tensor("iota", [P, MAXB, P], BF16, kind="ExternalInput")
    dinv_d = nc.dram_tensor("dinv", [P, W], F32, kind="ExternalInput")
    valid_d = nc.dram_tensor("valid", [P, W], F32, kind="ExternalInput")
    out_d = nc.dram_tensor("out", [P, W, C], F32, kind="ExternalOutput")

    ag_ins = [[nc.dram_tensor(f"ag_in{g}_{i}", [P, GW[g] * D], BF16)
               for g in range(G)] for i in range(2)]
    tables = [[nc.dram_tensor(f"table{g}_{i}", [NCORES * GW[g] * P, D], BF16,
                              addr_space="Shared") for g in range(G)]
              for i in range(2)]
    bn_in = nc.dram_tensor("bn_in", [P, 4], F32)
    bn_out = nc.dram_tensor("bn_out", [P, 4], F32, addr_space="Shared")

    CH = 7                                     # encoder windows per x-chunk

    with TileContext(nc) as tc, contextlib.ExitStack() as ctx:
        cp = ctx.enter_context(tc.tile_pool(name="const", bufs=1))
        big = ctx.enter_context(tc.tile_pool(name="big", bufs=2))
        htp = ctx.enter_context(tc.tile_pool(name="htp", bufs=2))

        nc.gpsimd.load_library(library_config.mlp)

        def cload(dram, shape, dtype, tag, src=None):
            t = cp.tile(shape, dtype, tag=tag, name=tag)
            nc.sync.dma_start(t[:], dram[:] if src is None else src)
            return t

        ident_t = cload(ident_d, [P, P], BF16, "ident")
        w1_t = cload(w1_d, [P, 2, D], BF16, "w1", src=w1_d[0])
        w2_t = cload(w2_d, [P, 2, D], BF16, "w2", src=w2_d[0])
        wm1_t = cload(wm1_d, [P, 2, D], BF16, "wm1", src=wm1_d[0])
        wm2_t = cload(wm2_d, [P, 2, C], BF16, "wm2", src=wm2_d[0])
        b1r_t = cload(b1r_d, [P, D], F32, "b1r")
        b2r_t = cload(b2r_d, [P, D], F32, "b2r")
        bcr_t = cload(bcr_d, [P, C], F32, "bcr")
        gam_t = cload(gam_d, [P, 2], F32, "gam")
        bet_t = cload(bet_d, [P, 2], F32, "bet")
        gidx_t = cload(gidx_d, [P, BTOT * 8], I16, "gidx")
        dl_t = cload(dl_d, [P, BTOT, 1], BF16, "dl")
        iota_t = cload(iota_d, [P, MAXB, P], BF16, "iota")
        dinv_t = cload(dinv_d, [P, W], F32, "dinv")
        valid_t = cload(valid_d, [P, W], F32, "valid")

        # persistent activations: ht slots rotate h0T -> h1T -> h2T -> h4T
        ht = [htp.tile([P, 2, NP], BF16, tag="ht", name=f"ht{i}")
              for i in range(3)]
        # hpre0 / hpre1 / h3 share one 2-deep rotation: h3 (layer-2 output)
        # reuses hpre0's buffer, whose last reader is layer 1's self-loop.
        hpre = [big.tile([P, W, D], BF16, tag="bigbuf", name=f"hpre{i}")
                for i in range(2)]
        h3 = big.tile([P, 2, NP], BF16, tag="bigbuf", name="h3")
        sumps = cp.tile([P, 2, W], F32, tag="sumps")
        sqps = cp.tile([P, 2, W], F32, tag="sqps")

        def send(li, g):
            nc.sync.dma_start(ag_ins[li][g][:],
                              hpre[li][:, GB[g]:GB[g + 1], :])
            nc.gpsimd.collective_compute(
                "AllGather", AX.bypass, ins=[ag_ins[li][g][:]],
                outs=[tables[li][g][:]], replica_groups=RG)

        send_at = {GB[g + 1] - 1: g for g in range(G)}

        # ── encoder: h0 = x @ lin_w (node-major) → transpose → ht[0],
        #    with conv1's pre-matmul interleaved per window
        with tc.tile_pool(name="encw", bufs=2) as wp, \
             tc.tile_pool(name="encp", bufs=2, space="PSUM") as pp, \
             tc.tile_pool(name="xtp", bufs=2) as xtp:
            wlin_t = []
            for k2 in range(K2E):
                t = cp.tile([P, 2, D], BF16, tag=f"wlin{k2}", name=f"wlin{k2}")
                nc.sync.dma_start(t[:], wlin_d[k2])
                wlin_t.append(t)
            for wc in range(_cdiv(W, CH)):
                ws, we = wc * CH, min(W, (wc + 1) * CH)
                xtc = []
                for k2 in range(K2E):
                    t = xtp.tile([P, 2, CH * P], BF16, tag=f"xtc{k2}",
                                 name=f"xtc{k2}_{wc}")
                    eng = nc.sync if k2 % 2 == 0 else nc.scalar
                    eng.dma_start(t[:, :, :(we - ws) * P],
                                  xt_d[k2][:, :, ws * P:we * P])
                    xtc.append(t)
                for w in range(ws, we):
                    lsl = slice((w - ws) * P, (w - ws + 1) * P)
                    sl = slice(w * P, (w + 1) * P)
                    ps = pp.tile([P, D], F32, tag="ps", name=f"eps{w}")
                    for k2 in range(K2E):
                        for r in range(2):
                            nc.tensor.matmul(
                                ps[:], xtc[k2][:, r, lsl], wlin_t[k2][:, r, :],
                                start=(k2 == 0 and r == 0),
                                stop=(k2 == K2E - 1 and r == 1))
                    hb = wp.tile([P, D], BF16, tag="hb", name=f"ehb{w}")
                    nc.vector.tensor_copy(hb[:], ps[:])
                    for r in range(2):
                        pt = pp.tile([P, P], BF16, tag="pt", name=f"ept{w}_{r}")
                        nc.tensor.transpose(pt[:], hb[:, r * P:(r + 1) * P],
                                            ident_t[:])
                        nc.vector.tensor_copy(ht[0][:, r, sl], pt[:])
                    ps2 = pp.tile([P, D], F32, tag="ps_pre", name=f"pre0_{w}")
                    for r in range(2):
                        nc.tensor.matmul(ps2[:], ht[0][:, r, sl], w1_t[:, r, :],
                                         start=(r == 0), stop=(r == 1))
                    nc.scalar.activation(hpre[0][:, w, :], ps2[:], AFT.Copy,
                                         scale=dinv_t[:, w:w + 1])
                    if w in send_at:
                        send(0, send_at[w])

        # ── conv layers (layer li consumes tableA/B[li]; the next stage's
        #    pre-matmul + halo send are interleaved into this layer's loop)
        for li in range(2):
            HT_out = ht[li + 1]
            br = b1r_t if li == 0 else b2r_t
            with tc.tile_pool(name=f"cw{li}", bufs=3) as wp, \
                 tc.tile_pool(name=f"cp{li}", bufs=2, space="PSUM") as pp:

                chunks = {}
                qc = [0]

                def _get_chunk(hs, ci, chunks=chunks, wp=wp, li=li):
                    key = (hs, ci)
                    if key in chunks:
                        return chunks[key]
                    nstream = NST[hs]
                    base_blk = SBASE[hs] + ci * CB
                    nblk = min(CB, nstream - ci * CB)
                    nn = nblk * P
                    gt = wp.tile([P, CB, D], BF16, tag=f"gt{hs}",
                                 name=f"gt{li}_{hs}_{ci}", bufs=5)
                    stt = wp.tile([P, CB, P], FP8, tag=f"st{hs}",
                                  name=f"st{li}_{hs}_{ci}", bufs=5)
                    tb = tables[li][hs][:]
                    nc.gpsimd.dma_gather(
                        gt[:, :nblk, :], tb,
                        gidx_t[:, base_blk * 8:(base_blk + nblk) * 8],
                        nn, nn, D, single_packet=True,
                        queue_num=qc[0] % 4)
                    qc[0] += 1
                    # build the 0/1 selection block on-device: S[e,b,j] =
                    # (j == dst_lane[e,b]); padding slots have dl = -1.
                    in0, in1 = bass.broadcast_tensor_aps(
                        iota_t[:, :nblk, :],
                        dl_t[:, base_blk:base_blk + nblk, :])
                    nc.vector.tensor_tensor(stt[:, :nblk, :], in0, in1,
                                            op=AX.is_equal)
                    chunks[key] = (gt, stt)
                    return chunks[key]

                def chunk_spans(off, nblk):
                    out = []
                    b = off
                    while b < off + nblk:
                        ci = b // CB
                        b1 = min(off + nblk, (ci + 1) * CB)
                        out.append((ci, b - ci * CB, b1 - ci * CB))
                        b = b1
                    return out

                # issue a few chunks of the early-landing streams before
                # the last group's first gather (which waits on its
                # AllGather) head-of-line blocks the in-order gpsimd
                for g in range(G - 1):
                    for ci in range(min(3 + g, _cdiv(NST[g], CB))):
                        _get_chunk(g, ci)

                for w in range(W):
                    sl = slice(w * P, (w + 1) * P)
                    pa = pp.tile([P, D], F32, tag="ps_agg", name=f"agg{li}_{w}")
                    spans = [(g, s) for g in range(G)
                             for s in chunk_spans(OFF[g][w], BLK[g][w])]
                    nmm = sum(s[2] - s[1] for _, s in spans)
                    # self-loop folded into psum: pa = hpre[w] + sum S.gt
                    nc.tensor.matmul(pa[:], ident_t[:], hpre[li][:, w, :],
                                     start=True, stop=False)
                    mi = 0
                    for hs, (ci, b0, b1) in spans:
                        gt, stt = _get_chunk(hs, ci)
                        for b in range(b0, b1):
                            nc.tensor.matmul(pa[:], stt[:, b, :], gt[:, b, :],
                                             start=False,
                                             stop=(mi == nmm - 1))
                            mi += 1
                    tf2 = wp.tile([P, D], F32, tag="tf2", name=f"tf2{li}_{w}",
                                   bufs=2)
                    nc.vector.scalar_tensor_tensor(
                        tf2[:], pa[:], dinv_t[:, w:w + 1], br[:],
                        op0=AX.mult, op1=AX.add)
                    hb = wp.tile([P, D], BF16, tag="hb2", name=f"chb{li}_{w}",
                                  bufs=2)
                    nc.scalar.activation(hb[:], tf2[:], AFT.Relu,
                                         scale=valid_t[:, w:w + 1])
                    for r in range(2):
                        pt = pp.tile([P, P], BF16, tag="pt",
                                     name=f"cpt{li}_{w}_{r}")
                        nc.tensor.transpose(pt[:], hb[:, r * P:(r + 1) * P],
                                            ident_t[:])
                        nc.vector.tensor_copy(HT_out[:, r, sl], pt[:])
                    if li == 0:
                        ps2 = pp.tile([P, D], F32, tag="ps_pre",
                                      name=f"pre1_{w}")
                        for r in range(2):
                            nc.tensor.matmul(ps2[:], HT_out[:, r, sl],
                                             w2_t[:, r, :],
                                             start=(r == 0), stop=(r == 1))
                        nc.scalar.activation(hpre[1][:, w, :], ps2[:], AFT.Copy,
                                             scale=dinv_t[:, w:w + 1])
                        if w in send_at:
                            send(1, send_at[w])
                    else:
                        # decoder mlp1 per window (feat-major) + BN stats
                        for fb in range(2):
                            pm = pp.tile([P, P], F32, tag=f"pm{fb}",
                                         name=f"pm{fb}_{w}")
                            for r in range(2):
                                nc.tensor.matmul(
                                    pm[:], wm1_t[:, r, fb * P:(fb + 1) * P],
                                    HT_out[:, r, sl],
                                    start=(r == 0), stop=(r == 1))
                            nc.vector.tensor_scalar(
                                h3[:, fb, sl], pm[:], 1.0, 0.0, op0=AX.mult,
                                op1=AX.add, accum_out=sumps[:, fb, w:w + 1])
                            scr = wp.tile([P, P], F32, tag=f"scr{fb}",
                                          name=f"scr{fb}_{w}", bufs=2)
                            nc.vector.scalar_tensor_tensor(
                                scr[:], h3[:, fb, sl], 1.0, h3[:, fb, sl],
                                op0=AX.mult, op1=AX.mult,
                                accum_out=sqps[:, fb, w:w + 1])

        # ── decoder: BN + relu + mlp2 + softmax (mlp1 ran inside layer 2)
        ht4 = htp.tile([P, 2, NP], BF16, tag="ht", name="ht4")
        with tc.tile_pool(name="dec", bufs=2) as wp, \
             tc.tile_pool(name="decp", bufs=2, space="PSUM") as pp, \
             tc.tile_pool(name="st1", bufs=1) as sp:
            sums = sp.tile([P, 2], F32, tag="sums")
            sqs = sp.tile([P, 2], F32, tag="sqs")
            for fb in range(2):
                nc.vector.reduce_sum(sums[:, fb:fb + 1], sumps[:, fb, :],
                                     axis=mybir.AxisListType.X)
                nc.vector.reduce_sum(sqs[:, fb:fb + 1], sqps[:, fb, :],
                                     axis=mybir.AxisListType.X)
            bnio = sp.tile([P, 4], F32, tag="bnio")
            nc.vector.tensor_copy(bnio[:, 0:2], sums[:])
            nc.vector.tensor_copy(bnio[:, 2:4], sqs[:])
            nc.sync.dma_start(bn_in[:], bnio[:])
            nc.gpsimd.collective_compute(
                "AllReduce", AX.add, ins=[bn_in[:]], outs=[bn_out[:]],
                replica_groups=RG)
            bns = sp.tile([P, 4], F32, tag="bns")
            nc.sync.dma_start(bns[:], bn_out[:])

            mu = sp.tile([P, 2], F32, tag="mu")
            nc.vector.tensor_scalar(mu[:], bns[:, 0:2], ninv, None, op0=AX.mult)
            msq = sp.tile([P, 2], F32, tag="msq")
            nc.vector.tensor_tensor(msq[:], mu[:], mu[:], op=AX.mult)
            var = sp.tile([P, 2], F32, tag="var")
            nc.vector.scalar_tensor_tensor(var[:], bns[:, 2:4], ninv, msq[:],
                                           op0=AX.mult, op1=AX.subtract)
            vae = sp.tile([P, 2], F32, tag="vae")
            nc.vector.tensor_scalar(vae[:], var[:], BN_EPS, None, op0=AX.add)
            sd = sp.tile([P, 2], F32, tag="sd")
            nc.scalar.activation(sd[:], vae[:], AFT.Sqrt)
            rstd = sp.tile([P, 2], F32, tag="rstd")
            nc.vector.reciprocal(rstd[:], sd[:])
            A = sp.tile([P, 2], F32, tag="A")
            nc.vector.tensor_tensor(A[:], rstd[:], gam_t[:], op=AX.mult)
            tb = sp.tile([P, 2], F32, tag="tb")
            nc.vector.tensor_tensor(tb[:], mu[:], A[:], op=AX.mult)
            B = sp.tile([P, 2], F32, tag="B")
            nc.vector.tensor_tensor(B[:], bet_t[:], tb[:], op=AX.subtract)

            lg = sp.tile([P, W, C], F32, tag="lg")
            ex = sp.tile([P, W, C], F32, tag="ex")
            rs = sp.tile([P, W], F32, tag="rs")
            ri = sp.tile([P, W], F32, tag="ri")
            outst = sp.tile([P, W, C], F32, tag="outst")
            TW = 12
            for wc0 in range(0, W, TW):
                wc1 = min(W, wc0 + TW)
                csl = slice(wc0 * P, wc1 * P)
                for fb in range(2):
                    nc.scalar.activation(ht4[:, fb, csl], h3[:, fb, csl],
                                         AFT.Relu, bias=B[:, fb:fb + 1],
                                         scale=A[:, fb:fb + 1])
                for w in range(wc0, wc1):
                    sl = slice(w * P, (w + 1) * P)
                    pl = pp.tile([P, C], F32, tag="ps_lg", name=f"plg{w}")
                    for r in range(2):
                        nc.tensor.matmul(pl[:], ht4[:, r, sl], wm2_t[:, r, :],
                                         start=(r == 0), stop=(r == 1))
                    nc.vector.scalar_tensor_tensor(lg[:, w, :], pl[:], 1.0,
                                                   bcr_t[:],
                                                   op0=AX.mult, op1=AX.add)
                nc.scalar.activation(
                    ex[:, wc0:wc1, :].rearrange("p w c -> p (w c)"),
                    lg[:, wc0:wc1, :].rearrange("p w c -> p (w c)"), AFT.Exp)
                nc.vector.reduce_sum(rs[:, wc0:wc1], ex[:, wc0:wc1, :],
                                     axis=mybir.AxisListType.X)
                nc.vector.reciprocal(ri[:, wc0:wc1], rs[:, wc0:wc1])
                for w in range(wc0, wc1):
                    nc.vector.tensor_scalar(outst[:, w, :], ex[:, w, :],
                                            ri[:, w:w + 1], None, op0=AX.mult)
                nc.sync.dma_start(out_d[:, wc0:wc1, :], outst[:, wc0:wc1, :])

    nc.compile()
    return nc


# ──────────────────────────────────────────────────────────────────────
# NTFF profiling shim (only needed when TRACE)
# ──────────────────────────────────────────────────────────────────────

def _install_hook():
    if "antenv.axon_hooks" in sys.modules:
        return
    so_path = "/opt/axon/libaxon_pjrt.so"
    holder = {"hook": None}
    mod = types.ModuleType("antenv.axon_hooks")
    mod.set_axon_ntff_profile_hook = lambda h: holder.__setitem__("hook", h)
    mod.get_axon_ntff_profile_hook = lambda: holder["hook"]
    sys.modules["antenv.axon_hooks"] = mod
    try:
        import antenv
        antenv.axon_hooks = mod
    except ImportError:
        pass
    try:
        lib = ctypes.CDLL(so_path)
        lib.axon_start_nrt_profile.argtypes = [ctypes.POINTER(ctypes.c_int64),
                                               ctypes.c_size_t]
        lib.axon_start_nrt_profile.restype = ctypes.c_int64
        lib.axon_stop_nrt_profile.argtypes = [ctypes.c_char_p]
        lib.axon_stop_nrt_profile.restype = ctypes.c_int64

        @contextlib.contextmanager
        def _hook(output_dir, device_ids):
            import jax
            jax.devices()
            if device_ids:
                ids = (ctypes.c_int64 * len(device_ids))(*device_ids)
                rc = lib.axon_start_nrt_profile(ids, len(device_ids))
            else:
                rc = lib.axon_start_nrt_profile(None, 0)
            if rc != 0:
                raise RuntimeError(f"axon_start_nrt_profile rc={rc}")
            try:
                yield
            finally:
                nf = lib.axon_stop_nrt_profile(str(output_dir).encode())
                if nf < 0:
                    raise RuntimeError(f"axon_stop_nrt_profile rc={nf}")

        holder["hook"] = _hook
    except OSError:
        pass


# ──────────────────────────────────────────────────────────────────────
# entry point
# ──────────────────────────────────────────────────────────────────────

def kernel(x, edge_index, lin_w, conv1_w, conv1_b, conv2_w, conv2_b,
           mlp1_w, mlp1_b, bn_gamma, bn_beta, mlp2_w, mlp2_b):
    x = np.asarray(x, np.float32)
    n, g = x.shape
    D = int(np.asarray(lin_w).shape[1])
    C = int(np.asarray(mlp2_w).shape[1])
    KENC = _cdiv(g, 256) * 256
    K2E = KENC // 256

    plan = _plan(n, np.asarray(edge_index))
    NP, W, BTOT = plan["NP"], plan["W"], plan["BTOT"]

    key = (n, g, D, C, NP,
           tuple(tuple(b) for b in plan["BLK"]), tuple(plan["GB"]))
    if key not in _CACHE:
        _CACHE[key] = _build(1.0 / float(n), NP, W, plan["GB"], plan["GW"],
                             plan["BLK"], plan["OFF"], plan["NST"],
                             plan["SBASE"], BTOT, plan["MAXB"],
                             K2E, D, C)
    nc = _CACHE[key]

    shared = {
        "wlin": _pack_k(lin_w, KENC),
        "w1": _pack_k(conv1_w, D),
        "w2": _pack_k(conv2_w, D),
        "wm1": _pack_k(mlp1_w, D),
        "wm2": _pack_k(mlp2_w, D),
        "b1r": np.ascontiguousarray(
            np.broadcast_to(np.asarray(conv1_b, np.float32), (P, D))),
        "b2r": np.ascontiguousarray(
            np.broadcast_to(np.asarray(conv2_b, np.float32), (P, D))),
        "bcr": np.ascontiguousarray(
            np.broadcast_to(np.asarray(mlp2_b, np.float32), (P, C))),
        "gam": np.ascontiguousarray(
            np.asarray(bn_gamma, np.float32).reshape(2, P).T),
        "bet": np.ascontiguousarray(
            np.asarray(bn_beta, np.float32).reshape(2, P).T),
        "ident": np.eye(P, dtype=np.float32).astype(ml_dtypes.bfloat16),
        "iota": np.ascontiguousarray(np.broadcast_to(
            np.arange(P, dtype=np.float32), (P, plan["MAXB"], P))).astype(
                ml_dtypes.bfloat16),
    }

    in_maps = []
    for c in range(NCORES):
        xs = x[c * NP:(c + 1) * NP]
        if xs.shape[0] < NP:
            xs = np.vstack([xs, np.zeros((NP - xs.shape[0], g), np.float32)])
        xt = _pack_k(np.ascontiguousarray(xs.T), KENC)
        in_maps.append(dict(shared,
                            xt=xt,
                            gidx=plan["gidx"][c],
                            dl=plan["dl"][c],
                            dinv=plan["dinv"][c],
                            valid=plan["valid"][c]))

    if TRACE:
        _install_hook()
        res = run_bass_kernel_spmd(nc, in_maps, core_ids=list(range(NCORES)),
                                   trace=True, **TRACE_KW)
        LAST["exec_time_ns"] = res.exec_time_ns
        LAST["res"] = res
    else:
        res = run_bass_kernel_spmd(nc, in_maps, core_ids=list(range(NCORES)))

    parts = []
    for c in range(NCORES):
        o = np.asarray(res.results[c]["out"])            # [P, W, C]
        parts.append(np.ascontiguousarray(o.transpose(1, 0, 2)).reshape(NP, C))
    return np.concatenate(parts, axis=0)[:n].astype(np.float32)

